# revision 13
# baseline (speedup 1.0000x reference)
"""Expert-parallel batched-expert FFN kernel for Trainium2 (8 NeuronCores).

Reference computation (per expert e):
    y = relu(x[e] @ fc1_w[e] + fc1_b[e]) @ fc2_w[e] + fc2_b[e]

Sharding: E=8 experts, one expert per core (expert parallel, no collectives).

Per-core algorithm (T=2048 tokens, D=1024, H=4096):
  - x is fed to the device as fp16 and transposed upfront on the PE
    (DMA transpose only handles 2-byte dtypes) into xT [D, T] chunks.
  - FC1 produces yT [H, T] so FC2 can consume it as the stationary operand
    directly; both weight matrices stream (once) from DRAM in natural
    row-major layout on the scalar-engine HWDGE ring; x/out use the sync
    ring so the streams don't serialize behind each other.
  - Stream over H in blocks of 512; FC2 accumulates each block's 4 k-tiles
    in PSUM, then a DVE add folds the partial into the fp16 SBUF
    accumulator (bias b2 is folded into the first add).
  - Matmul operands are fp16: inputs round to ~2^-11 relative; all
    accumulation is fp32 in PSUM; the cross-block accumulator is fp16
    (adds ~8e-4 L2 error, well within budget), stored as fp16 and upcast
    on the host.
  - Ramp: HAM ignores transpose-mode PE work, so warm-up uses real
    dependency-free matmuls; block 0's FC1 is interleaved with the x
    transposes chunk-by-chunk so real matmuls start as soon as the first
    512 tokens have landed, keeping the PE dense through the DMA ramp.
  - Tail: each output tile's store is emitted right after its final
    accumulator add so stores overlap the last block's compute.
"""

from contextlib import ExitStack

import numpy as np

import concourse.bass as bass
import concourse.bacc as bacc
import concourse.mybir as mybir
import concourse.tile as tile
from concourse.bass_utils import run_bass_kernel_spmd
from concourse.masks import make_identity

E, T, D, H = 8, 2048, 1024, 4096
NCORES = 8
HB = 512           # h per stream block
FP = mybir.dt.float32
FP16 = mybir.dt.float16
RELU = mybir.ActivationFunctionType.Relu

N_BLK = H // HB                # 8
N_HI = HB // 128               # 4  h-tiles per block
N_KI = D // 128                # 8  k-tiles for FC1
N_TI = T // 128                # 16 token tiles
N_DC = D // 512                # 2  512-col chunks of D
N_C4 = T // 512                # 4  512-token chunks


def _emit_kernel(tc, out, x, w1, b1, w2, b2):
    nc = tc.nc
    with ExitStack() as ctx:
        singles = ctx.enter_context(tc.tile_pool(name="singles", bufs=1))
        xload = ctx.enter_context(tc.tile_pool(name="xload", bufs=3))
        xt_pool = ctx.enter_context(tc.tile_pool(name="xt", bufs=1))
        yt_pool = ctx.enter_context(tc.tile_pool(name="yt", bufs=N_HI))
        acc_pool = ctx.enter_context(tc.tile_pool(name="acc", bufs=1))
        w1_pool = ctx.enter_context(tc.tile_pool(name="w1", bufs=4))
        w2_pool = ctx.enter_context(tc.tile_pool(name="w2", bufs=3))
        psum = ctx.enter_context(tc.tile_pool(name="psum", bufs=4, space="PSUM"))

        ident = singles.tile([128, 128], FP16)
        make_identity(nc, ident)

        # b1 [1, H] -> [128, H//128] with [p, hi] = b1[hi*128 + p]
        b1t = singles.tile([128, H // 128], FP)
        nc.scalar.dma_start(out=b1t, in_=b1.rearrange("o (h p) -> (o p) h", p=128))

        # b2 [1, D] broadcast across partitions -> [128, D]
        b2b = singles.tile([128, D], FP)
        b2_bcast = bass.AP(tensor=b2.tensor, offset=b2.offset,
                           ap=[[0, 128]] + [list(b2.ap[-1])])
        nc.scalar.dma_start(out=b2b, in_=b2_bcast)

        # w1 viewed so a [p, k, m] DMA gives lhsT tiles: [d%128, d//128, h]
        w1v = w1.rearrange("(k p) h -> p k h", p=128)

        # HAM warm-up: REAL dependency-free matmuls (transpose-mode PE work
        # does not count as PE-busy for the HAM clock gate), issued at t=0 so
        # the PE clock reaches 8/8 by ~3.5us and stays there once real
        # matmuls take over.
        wtile = singles.tile([128, 128], FP16)
        nc.vector.memset(wtile, 0.0)

        def emit_warm(n):
            for i in range(n):
                pt = psum.tile([128, 128], FP, tag="psB",
                               name=f"wu{i}")
                nc.tensor.matmul(pt, lhsT=wtile, rhs=wtile,
                                 start=True, stop=True)

        emit_warm(56)

        # one 1MB DMA per w1 block: DMA issue costs ~0.7us of engine time
        # each, so batching matters for the ramp
        def load_w1_block(b):
            wp = w1_pool.tile([128, N_KI, HB], FP16, tag="w1",
                              name=f"w1p{b}")
            nc.scalar.dma_start(out=wp, in_=w1v[:, :, b * HB:(b + 1) * HB])
            return wp

        # xT[k][c4] = x[c4-chunk, k-tile].T
        xT = [[xt_pool.tile([128, 512], FP16, tag=f"xt{k}_{c4}",
                            name=f"xT{k}_{c4}")
               for c4 in range(N_C4)] for k in range(N_KI)]

        def emit_xpose(c4):
            # one 1MB DMA per 512-token chunk, alternating rings
            xs = xload.tile([128, 4, D], FP16, tag="xload", name=f"xs{c4}")
            # all x on the sync HW ring: the gpsimd software-DGE queue only
            # sustains ~30 GB/s and stalled the ramp
            ring = nc.sync
            ring.dma_start(
                out=xs,
                in_=x[c4 * 512:(c4 + 1) * 512, :].rearrange(
                    "(r p) d -> p r d", p=128))
            for col in range(4):
                ti = c4 * 4 + col
                for k in range(N_KI):
                    pt = psum.tile([128, 128], FP16, tag="psB",
                                   name=f"psx{ti}_{k}")
                    nc.tensor.transpose(out=pt,
                                        in_=xs[:, col, k * 128:(k + 1) * 128],
                                        identity=ident)
                    nc.vector.tensor_copy(
                        xT[k][c4][:, col * 128:(col + 1) * 128], pt)

        accs = [[acc_pool.tile([128, 512], FP16, tag=f"acc{ti}_{dc}",
                               name=f"acc{ti}_{dc}")
                 for dc in range(N_DC)] for ti in range(N_TI)]

        # The DMA ramp only sustains ~150-200 GB/s (chip-shared across the 8
        # cores), so x's 4MB cannot land fast enough for a single FC1 block
        # (27us of PE work) to cover it.  Interleave the FC1 of the first
        # RAMP_BLKS blocks across the c4 token chunks: 3x the PE work per MB
        # of x keeps the PE dense (and the HAM clock warm) through the ramp.
        RAMP_BLKS = 3
        w1p_ramp = [load_w1_block(b) for b in range(RAMP_BLKS)]
        yTb_ramp = [[yt_pool.tile([128, T], FP16, tag="yt", bufs=3 * N_HI,
                                  name=f"yT{b}_{i}")
                     for i in range(N_HI)] for b in range(RAMP_BLKS)]
        for c4 in range(N_C4):
            emit_xpose(c4)
            for b in range(RAMP_BLKS):
                for hi in range(N_HI):
                    h_abs = b * N_HI + hi
                    pt = psum.tile([128, 512], FP, tag="psA",
                                   name=f"psfc1_{b}_{hi}_{c4}")
                    for ki in range(N_KI):
                        nc.tensor.matmul(
                            pt,
                            lhsT=w1p_ramp[b][:, ki, hi * 128:(hi + 1) * 128],
                            rhs=xT[ki][c4],
                            start=(ki == 0), stop=(ki == N_KI - 1))
                    nc.scalar.activation(
                        out=yTb_ramp[b][hi][:, c4 * 512:(c4 + 1) * 512],
                        in_=pt,
                        func=RELU, bias=b1t[:, h_abs:h_abs + 1], scale=1.0)

        w1p = None
        yTb = None
        for b in range(N_BLK):
            # ---- FC1: yT block [HB, T] = relu(w1.T @ xT + b1) ----
            if b < RAMP_BLKS:
                yTb = yTb_ramp[b]
            else:
                w1p = load_w1_block(b)
                yTb = [yt_pool.tile([128, T], FP16, tag="yt", bufs=3 * N_HI,
                                    name=f"yT{b}_{i}")
                       for i in range(N_HI)]
                for hi in range(N_HI):
                    h_abs = b * N_HI + hi
                    for half in range(N_C4 // 2):
                        pts = [psum.tile([128, 512], FP, tag="psA",
                                         name=f"psfc1_{b}_{hi}_{half}_{t}")
                               for t in range(2)]
                        for ki in range(N_KI):
                            for tch in range(2):
                                nc.tensor.matmul(
                                    pts[tch],
                                    lhsT=w1p[:, ki, hi * 128:(hi + 1) * 128],
                                    rhs=xT[ki][half * 2 + tch],
                                    start=(ki == 0), stop=(ki == N_KI - 1))
                        for tch in range(2):
                            c4 = half * 2 + tch
                            nc.scalar.activation(
                                out=yTb[hi][:, c4 * 512:(c4 + 1) * 512],
                                in_=pts[tch],
                                func=RELU, bias=b1t[:, h_abs:h_abs + 1],
                                scale=1.0)

            # ---- FC2 partial: acc += yTb.T @ w2[block] ----
            w2t = w2_pool.tile([128, N_HI, D], FP16, tag="w2",
                               name=f"w2t{b}")
            nc.scalar.dma_start(
                out=w2t,
                in_=w2[b * HB:(b + 1) * HB, :].rearrange(
                    "(r p) d -> p r d", p=128))

            for ti in range(N_TI):
                pts = [psum.tile([128, 512], FP, tag="psB",
                                 name=f"psfc2_{b}_{ti}_{d}")
                       for d in range(N_DC)]
                for hk in range(N_HI):
                    for dc in range(N_DC):
                        nc.tensor.matmul(
                            pts[dc],
                            lhsT=yTb[hk][:, ti * 128:(ti + 1) * 128],
                            rhs=w2t[:, hk, dc * 512:(dc + 1) * 512],
                            start=(hk == 0), stop=(hk == N_HI - 1))
                for dc in range(N_DC):
                    if b == 0:
                        nc.vector.tensor_add(
                            accs[ti][dc], pts[dc],
                            b2b[:, dc * 512:(dc + 1) * 512])
                    else:
                        nc.vector.tensor_add(
                            accs[ti][dc], accs[ti][dc], pts[dc])
                    if b == N_BLK - 1:
                        # store as soon as this tile's accumulation is done
                        nc.sync.dma_start(
                            out=out[ti * 128:(ti + 1) * 128,
                                    dc * 512:(dc + 1) * 512],
                            in_=accs[ti][dc])


def build_module():
    nc = bacc.Bacc("TRN2", target_bir_lowering=False, debug=False)
    x = nc.dram_tensor("x", [T, D], FP16, kind="ExternalInput").ap()
    w1 = nc.dram_tensor("fc1_w", [D, H], FP16, kind="ExternalInput").ap()
    b1 = nc.dram_tensor("fc1_b", [1, H], FP, kind="ExternalInput").ap()
    w2 = nc.dram_tensor("fc2_w", [H, D], FP16, kind="ExternalInput").ap()
    b2 = nc.dram_tensor("fc2_b", [1, D], FP, kind="ExternalInput").ap()
    out = nc.dram_tensor("out", [T, D], FP16, kind="ExternalOutput").ap()
    with tile.TileContext(nc) as tc:
        _emit_kernel(tc, out, x, w1, b1, w2, b2)
    nc.compile()
    return nc


_CACHED = None


def kernel(x, fc1_w, fc1_b, fc2_w, fc2_b, _trace=False, _trace_cores=None):
    global _CACHED
    if _CACHED is None:
        _CACHED = build_module()
    nc = _CACHED

    x = np.ascontiguousarray(np.asarray(x, dtype=np.float32).astype(np.float16))
    fc1_w = np.ascontiguousarray(
        np.asarray(fc1_w, dtype=np.float32).astype(np.float16))
    fc1_b = np.ascontiguousarray(np.asarray(fc1_b, dtype=np.float32))
    fc2_w = np.ascontiguousarray(
        np.asarray(fc2_w, dtype=np.float32).astype(np.float16))
    fc2_b = np.ascontiguousarray(np.asarray(fc2_b, dtype=np.float32))

    in_maps = [
        {
            "x": x[e],
            "fc1_w": fc1_w[e],
            "fc1_b": fc1_b[e],
            "fc2_w": fc2_w[e],
            "fc2_b": fc2_b[e],
        }
        for e in range(E)
    ]
    kw = {}
    if _trace:
        kw = dict(trace=True,
                  trace_cores=_trace_cores if _trace_cores is not None else [0])
    res = run_bass_kernel_spmd(nc, in_maps, core_ids=list(range(NCORES)), **kw)
    out = np.stack([res.results[e]["out"].astype(np.float32)
                    for e in range(E)], axis=0)
    if _trace:
        return out, res
    return out


# revision 14
# speedup vs baseline: 1.1703x; 1.1703x over previous
"""Expert-parallel batched-expert FFN kernel for Trainium2 — Strassen FC1.

Reference computation (per expert e):
    y = relu(x[e] @ fc1_w[e] + fc1_b[e]) @ fc2_w[e] + fc2_b[e]

Sharding: E=8 experts, one expert per core (expert parallel, no collectives).

Per-core algorithm (T=2048 tokens, D=1024, H=4096), fp16 operands:
  - Tokens are processed in two halves of 1024.  Within a half, FC1 is
    computed with one level of Strassen-Winograd: A = x-half [1024, 1024]
    split into [512, 512] blocks, B = w1 [1024, 4096] into [512, 2048]
    blocks.  The 7 B-side operands (B11, B21, B22, T4, T1, T2, T3) are
    precomputed on the host and streamed; the 4 A-side operands S1..S4 are
    built on the DVE from the transposed x tiles; the 7 products run on the
    PE (7/8 of the classic MAC count); the C-quadrant assembly (7 adds) runs
    on the DVE out of PSUM, then ScalarE applies bias+relu producing the
    fp16 yT half [4096, 1024] resident in SBUF.
  - FC2 for a half accumulates its full 4096-deep contraction in PSUM
    (no partial-sum traffic): two sweeps (one per 512-wide output column
    chunk) of 8 concurrent [128, 512] PSUM banks; w2 columns stream per
    sweep.  Output tiles get bias via one DVE add and store as fp16.
  - Warm-up uses real dependency-free matmuls (HAM ignores transposes).
"""

from contextlib import ExitStack

import numpy as np

import concourse.bass as bass
import concourse.bacc as bacc
import concourse.mybir as mybir
import concourse.tile as tile
from concourse.bass_utils import run_bass_kernel_spmd
from concourse.masks import make_identity

E, T, D, H = 8, 2048, 1024, 4096
NCORES = 8
TH = T // 2                    # tokens per half
FP = mybir.dt.float32
FP16 = mybir.dt.float16
RELU = mybir.ActivationFunctionType.Relu

N_KI = D // 128                # 8  k-tiles of x
N_KJ = 4                       # k-tiles per Strassen d-block (512)
N_HT = 16                      # h-tiles per Strassen h-block (2048)
N_C4 = T // 512                # 4  512-token chunks
N_HK = H // 128                # 32 h k-tiles for FC2
N_TIL = TH // 128              # 8  token tiles per half
N_DC = D // 512                # 2


def _emit_kernel(tc, out, x, w1s, b1, w2, b2):
    nc = tc.nc
    with ExitStack() as ctx:
        singles = ctx.enter_context(tc.tile_pool(name="singles", bufs=1))
        xload = ctx.enter_context(tc.tile_pool(name="xload", bufs=2))
        xt_pool = ctx.enter_context(tc.tile_pool(name="xt", bufs=1))
        s_pool = ctx.enter_context(tc.tile_pool(name="spool", bufs=1))
        yt_pool = ctx.enter_context(tc.tile_pool(name="yt", bufs=1))
        w1s_pool = ctx.enter_context(tc.tile_pool(name="w1s", bufs=3))
        w2_pool = ctx.enter_context(tc.tile_pool(name="w2", bufs=3))
        us_pool = ctx.enter_context(tc.tile_pool(name="us", bufs=4))
        cs_pool = ctx.enter_context(tc.tile_pool(name="cs", bufs=5))
        os_pool = ctx.enter_context(tc.tile_pool(name="os", bufs=3))
        psum = ctx.enter_context(tc.tile_pool(name="psum", bufs=5, space="PSUM"))

        ident = singles.tile([128, 128], FP16)
        make_identity(nc, ident)

        # b1 [1, H] -> [128, H//128] with [p, hi] = b1[hi*128 + p]
        b1t = singles.tile([128, H // 128], FP)
        nc.scalar.dma_start(out=b1t, in_=b1.rearrange("o (h p) -> (o p) h", p=128))

        # b2 [1, D] broadcast across partitions -> [128, D]
        b2b = singles.tile([128, D], FP)
        b2_bcast = bass.AP(tensor=b2.tensor, offset=b2.offset,
                           ap=[[0, 128]] + [list(b2.ap[-1])])
        nc.scalar.dma_start(out=b2b, in_=b2_bcast)

        # w1s host layout [ht, p, kj, i, h'] -> per-ht loads are contiguous
        w1sv = w1s.rearrange("t (p r) -> t p r", p=128)

        # HAM warm-up with real matmuls
        wtile = singles.tile([128, 128], FP16)
        nc.vector.memset(wtile, 0.0)
        for i in range(56):
            pt = psum.tile([128, 128], FP, tag="psB", bufs=3, name=f"wu{i}")
            nc.tensor.matmul(pt, lhsT=wtile, rhs=wtile, start=True, stop=True)

        # xT[k][c4] = x[c4-chunk, k-tile].T
        xT = [[xt_pool.tile([128, 512], FP16, tag=f"xt{k}_{c4}",
                            name=f"xT{k}_{c4}")
               for c4 in range(N_C4)] for k in range(N_KI)]

        def emit_xpose(c4):
            # one 1MB DMA per 512-token chunk on the sync ring (keeping the
            # scalar ring free for the latency-critical w1s/w2 streams)
            xs = xload.tile([128, 4, D], FP16, tag="xload", name=f"xs{c4}")
            nc.sync.dma_start(
                out=xs,
                in_=x[c4 * 512:(c4 + 1) * 512, :].rearrange(
                    "(r p) d -> p r d", p=128))
            for col in range(4):
                ti = c4 * 4 + col
                for k in range(N_KI):
                    pt = psum.tile([128, 128], FP16, tag="psB", bufs=3,
                                   name=f"psx{ti}_{k}")
                    nc.tensor.transpose(
                        out=pt,
                        in_=xs[:, col, k * 128:(k + 1) * 128],
                        identity=ident)
                    nc.vector.tensor_copy(
                        xT[k][c4][:, col * 128:(col + 1) * 128], pt)

        def emit_sides(hf):
            # transposes + A-side Strassen operands for half hf
            c4a, c4b = 2 * hf, 2 * hf + 1
            emit_xpose(c4a)
            if hf == 0:
                # dependency-free PE filler while the second x chunk's DMA
                # lands (the c4b transposes would otherwise head-block the
                # PE queue)
                for i in range(48):
                    pt = psum.tile([128, 128], FP, tag="psB", bufs=3,
                                   name=f"wf{i}")
                    nc.tensor.matmul(pt, lhsT=wtile, rhs=wtile,
                                     start=True, stop=True)
            emit_xpose(c4b)
            s = [[s_pool.tile([128, 512], FP16, tag=f"s{si}_{hf}_{kj}",
                              name=f"s{si}_{hf}_{kj}") for kj in range(N_KJ)]
                 for si in (1, 2, 3, 4)]
            s1, s2, s3, s4 = s
            # s1 first: products needing it (M5) run before M6/M7/M3
            for kj in range(N_KJ):
                nc.vector.tensor_add(s1[kj], xT[kj][c4b], xT[4 + kj][c4b])
            for kj in range(N_KJ):
                nc.vector.tensor_sub(s2[kj], s1[kj], xT[kj][c4a])
            for kj in range(N_KJ):
                nc.vector.tensor_sub(s3[kj], xT[kj][c4a], xT[kj][c4b])
            for kj in range(N_KJ):
                nc.vector.tensor_sub(s4[kj], xT[4 + kj][c4a], s2[kj])
            return s1, s2, s3, s4

        wp_cache = {}

        def wpt(hf, ht):
            k = (hf, ht)
            if k not in wp_cache:
                wp = w1s_pool.tile([128, N_KJ, 7, 128], FP16, tag="w1s",
                                   name=f"wp{hf}_{ht}")
                nc.scalar.dma_start(out=wp, in_=w1sv[ht])
                wp_cache[k] = wp
            return wp_cache[k]

        wpt(0, 0)   # first weight chunk ahead of the x stream on scalar
        sides = emit_sides(0)
        for hf in range(2):
            c4a, c4b = 2 * hf, 2 * hf + 1
            s1, s2, s3, s4 = sides

            # rhs tiles per product (index 1..7), per kj
            rhs_of = {
                1: [xT[kj][c4a] for kj in range(N_KJ)],
                2: [xT[4 + kj][c4a] for kj in range(N_KJ)],
                3: s4,
                4: [xT[4 + kj][c4b] for kj in range(N_KJ)],
                5: s1,
                6: s2,
                7: s3,
            }

            yth = [yt_pool.tile([128, TH], FP16, tag=f"yth{ht}",
                                name=f"yth{hf}_{ht}") for ht in range(N_HK)]

            w2_cache = {}

            def w2t(dc, hg, key):
                k = (dc, hg, key)
                if k not in w2_cache:
                    wt = w2_pool.tile([128, 4, 512], FP16, tag="w2",
                                      name=f"w2t{hf}_{dc}_{hg}_{key}")
                    nc.scalar.dma_start(
                        out=wt,
                        in_=w2[hg * 512:(hg + 1) * 512,
                               dc * 512:(dc + 1) * 512].rearrange(
                                   "(r p) d -> p r d", p=128))
                    w2_cache[k] = wt
                return w2_cache[k]

            for ht in range(N_HT):
                if ht == 11:
                    w2t(0, 0, f"{hf}_0")   # prefetch first sweep chunk
                if ht == 14:
                    w2t(0, 1, f"{hf}_0")
                wp = wpt(hf, ht)

                def product(i_prod, nm):
                    mt = psum.tile([128, 512], FP, tag="psA",
                                   name=f"m{nm}_{hf}_{ht}")
                    for kj in range(N_KJ):
                        nc.tensor.matmul(
                            mt,
                            lhsT=wp[:, kj, i_prod - 1, :],
                            rhs=rhs_of[i_prod][kj],
                            start=(kj == 0), stop=(kj == N_KJ - 1))
                    return mt

                m1 = product(1, "1")
                m2 = product(2, "2")
                m4 = product(4, "4")
                m5 = product(5, "5")
                m6 = product(6, "6")

                # M1 is read twice; copy to SBUF on ScalarE so every DVE
                # tensor_tensor touches at most one PSUM bank
                m1c = us_pool.tile([128, 512], FP, tag="us", name=f"m1c{hf}_{ht}")
                nc.scalar.activation(out=m1c, in_=m1,
                                     func=mybir.ActivationFunctionType.Copy,
                                     scale=1.0)

                c11 = cs_pool.tile([128, 512], FP16, tag="cs", name=f"c11_{hf}_{ht}")
                u2 = us_pool.tile([128, 512], FP, tag="us", name=f"u2_{hf}_{ht}")
                u3 = us_pool.tile([128, 512], FP, tag="us", name=f"u3_{hf}_{ht}")
                u4 = us_pool.tile([128, 512], FP, tag="us", name=f"u4_{hf}_{ht}")
                nc.vector.tensor_add(c11, m1c, m2)
                nc.vector.tensor_add(u2, m1c, m6)
                m7 = product(7, "7")
                nc.vector.tensor_add(u3, u2, m7)
                nc.vector.tensor_add(u4, u2, m5)

                c21 = cs_pool.tile([128, 512], FP16, tag="cs", name=f"c21_{hf}_{ht}")
                nc.vector.tensor_sub(c21, u3, m4)
                m3 = product(3, "3")
                c12 = cs_pool.tile([128, 512], FP16, tag="cs", name=f"c12_{hf}_{ht}")
                nc.vector.tensor_add(c12, u4, m3)
                c22 = cs_pool.tile([128, 512], FP16, tag="cs", name=f"c22_{hf}_{ht}")
                nc.vector.tensor_add(c22, u3, m5)

                # bias + relu -> yT half tiles
                nc.scalar.activation(out=yth[ht][:, 0:512], in_=c11,
                                     func=RELU, bias=b1t[:, ht:ht + 1], scale=1.0)
                nc.scalar.activation(out=yth[ht][:, 512:1024], in_=c21,
                                     func=RELU, bias=b1t[:, ht:ht + 1], scale=1.0)
                nc.scalar.activation(out=yth[16 + ht][:, 0:512], in_=c12,
                                     func=RELU,
                                     bias=b1t[:, 16 + ht:17 + ht], scale=1.0)
                nc.scalar.activation(out=yth[16 + ht][:, 512:1024], in_=c22,
                                     func=RELU,
                                     bias=b1t[:, 16 + ht:17 + ht], scale=1.0)

            if hf == 0:
                sides = emit_sides(1)

            # ---- FC2 for this half: full contraction in PSUM ----
            # one sweep per 512-wide output chunk; 8 concurrent PSUM banks.
            # The very last sweep is split into two 4-til half-sweeps so the
            # final stores overlap the remaining matmuls instead of draining
            # ~1MB after the PE goes idle.
            def sweep(dc, tils, key):
                pts = {}
                for j, til in enumerate(tils):
                    tag = "psA" if j < 5 else "psB"
                    kw = dict(bufs=3) if j >= 5 else {}
                    pts[til] = psum.tile([128, 512], FP, tag=tag,
                                         name=f"psfc2_{key}_{til}", **kw)
                for hg in range(N_HK // 4):
                    wt = w2t(dc, hg, key)
                    for r in range(4):
                        hk = hg * 4 + r
                        for til in tils:
                            nc.tensor.matmul(
                                pts[til],
                                lhsT=yth[hk][:, til * 128:(til + 1) * 128],
                                rhs=wt[:, r, :],
                                start=(hk == 0), stop=(hk == N_HK - 1))
                for til in tils:
                    os = os_pool.tile([128, 512], FP16, tag="os",
                                      name=f"os_{key}_{til}")
                    nc.vector.tensor_add(os, pts[til],
                                         b2b[:, dc * 512:(dc + 1) * 512])
                    ti_abs = hf * N_TIL + til
                    ring = nc.sync if til % 2 == 0 else nc.scalar
                    ring.dma_start(
                        out=out[ti_abs * 128:(ti_abs + 1) * 128,
                                dc * 512:(dc + 1) * 512],
                        in_=os)

            sweep(0, list(range(N_TIL)), f"{hf}_0")
            sweep(1, list(range(N_TIL)), f"{hf}_1")


def build_module():
    nc = bacc.Bacc("TRN2", target_bir_lowering=False, debug=False)
    x = nc.dram_tensor("x", [T, D], FP16, kind="ExternalInput").ap()
    w1s = nc.dram_tensor("w1s", [16, 128 * 4 * 7 * 128], FP16,
                         kind="ExternalInput").ap()
    b1 = nc.dram_tensor("fc1_b", [1, H], FP, kind="ExternalInput").ap()
    w2 = nc.dram_tensor("fc2_w", [H, D], FP16, kind="ExternalInput").ap()
    b2 = nc.dram_tensor("fc2_b", [1, D], FP, kind="ExternalInput").ap()
    out = nc.dram_tensor("out", [T, D], FP16, kind="ExternalOutput").ap()
    with tile.TileContext(nc) as tc:
        _emit_kernel(tc, out, x, w1s, b1, w2, b2)
    nc.compile()
    return nc


_CACHED = None


def _host_w1s(w1_f32):
    """Host-side Strassen-Winograd B-operands: [B11,B21,B22,T4,T1,T2,T3]."""
    b11 = w1_f32[0:512, 0:2048]
    b12 = w1_f32[0:512, 2048:4096]
    b21 = w1_f32[512:1024, 0:2048]
    b22 = w1_f32[512:1024, 2048:4096]
    t1 = b12 - b11
    t2 = b22 - t1
    t3 = b22 - b12
    t4 = t2 - b21
    w = np.stack([b11, b21, b22, t4, t1, t2, t3], axis=1)  # [512, 7, 2048]
    # -> [ht, p, kj, i, h'] so each per-ht slice is one contiguous DMA
    w5 = w.reshape(4, 128, 7, 16, 128).transpose(3, 1, 0, 2, 4)
    return np.ascontiguousarray(
        w5.reshape(16, 128 * 4 * 7 * 128).astype(np.float16))


def kernel(x, fc1_w, fc1_b, fc2_w, fc2_b, _trace=False, _trace_cores=None):
    global _CACHED
    if _CACHED is None:
        _CACHED = build_module()
    nc = _CACHED

    x = np.ascontiguousarray(np.asarray(x, dtype=np.float32).astype(np.float16))
    fc1_w = np.asarray(fc1_w, dtype=np.float32)
    fc1_b = np.ascontiguousarray(np.asarray(fc1_b, dtype=np.float32))
    fc2_w = np.ascontiguousarray(
        np.asarray(fc2_w, dtype=np.float32).astype(np.float16))
    fc2_b = np.ascontiguousarray(np.asarray(fc2_b, dtype=np.float32))

    in_maps = [
        {
            "x": x[e],
            "w1s": _host_w1s(fc1_w[e]),
            "fc1_b": fc1_b[e],
            "fc2_w": fc2_w[e],
            "fc2_b": fc2_b[e],
        }
        for e in range(E)
    ]
    kw = {}
    if _trace:
        kw = dict(trace=True,
                  trace_cores=_trace_cores if _trace_cores is not None else [0])
    res = run_bass_kernel_spmd(nc, in_maps, core_ids=list(range(NCORES)), **kw)
    out = np.stack([res.results[e]["out"].astype(np.float32)
                    for e in range(E)], axis=0)
    if _trace:
        return out, res
    return out


# revision 15
# speedup vs baseline: 1.1913x; 1.0179x over previous
"""Expert-parallel batched-expert FFN kernel for Trainium2 — Strassen FC1.

Reference computation (per expert e):
    y = relu(x[e] @ fc1_w[e] + fc1_b[e]) @ fc2_w[e] + fc2_b[e]

Sharding: E=8 experts, one expert per core (expert parallel, no collectives).

Per-core algorithm (T=2048 tokens, D=1024, H=4096), fp16 operands:
  - Tokens are processed in two halves of 1024.  Within a half, FC1 is
    computed with one level of Strassen-Winograd: A = x-half [1024, 1024]
    split into [512, 512] blocks, B = w1 [1024, 4096] into [512, 2048]
    blocks.  The 7 B-side operands (B11, B21, B22, T4, T1, T2, T3) are
    precomputed on the host and streamed; the 4 A-side operands S1..S4 are
    built on the DVE from the transposed x tiles; the 7 products run on the
    PE (7/8 of the classic MAC count); the C-quadrant assembly (7 adds) runs
    on the DVE out of PSUM, then ScalarE applies bias+relu producing the
    fp16 yT half [4096, 1024] resident in SBUF.
  - FC2 for a half accumulates its full 4096-deep contraction in PSUM
    (no partial-sum traffic): two sweeps (one per 512-wide output column
    chunk) of 8 concurrent [128, 512] PSUM banks; w2 columns stream per
    sweep.  Output tiles get bias via one DVE add and store as fp16.
  - Warm-up uses real dependency-free matmuls (HAM ignores transposes).
"""

from contextlib import ExitStack

import numpy as np

import concourse.bass as bass
import concourse.bacc as bacc
import concourse.mybir as mybir
import concourse.tile as tile
from concourse.bass_utils import run_bass_kernel_spmd
from concourse.masks import make_identity

E, T, D, H = 8, 2048, 1024, 4096
NCORES = 8
TH = T // 2                    # tokens per half
FP = mybir.dt.float32
FP16 = mybir.dt.float16
RELU = mybir.ActivationFunctionType.Relu

N_KI = D // 128                # 8  k-tiles of x
N_KJ = 4                       # k-tiles per Strassen d-block (512)
N_HT = 16                      # h-tiles per Strassen h-block (2048)
N_C4 = T // 512                # 4  512-token chunks
N_HK = H // 128                # 32 h k-tiles for FC2
N_TIL = TH // 128              # 8  token tiles per half
N_DC = D // 512                # 2


B2_IS_ZERO = False


def _emit_kernel(tc, out, x, w1s, b1, w2, b2):
    nc = tc.nc
    with ExitStack() as ctx:
        singles = ctx.enter_context(tc.tile_pool(name="singles", bufs=1))
        xload = ctx.enter_context(tc.tile_pool(name="xload", bufs=2))
        xt_pool = ctx.enter_context(tc.tile_pool(name="xt", bufs=1))
        s_pool = ctx.enter_context(tc.tile_pool(name="spool", bufs=1))
        yt_pool = ctx.enter_context(tc.tile_pool(name="yt", bufs=1))
        w1s_pool = ctx.enter_context(tc.tile_pool(name="w1s", bufs=3))
        w2_pool = ctx.enter_context(tc.tile_pool(name="w2", bufs=3))
        us_pool = ctx.enter_context(tc.tile_pool(name="us", bufs=4))
        cs_pool = ctx.enter_context(tc.tile_pool(name="cs", bufs=5))
        os_pool = ctx.enter_context(tc.tile_pool(name="os", bufs=3))
        psum = ctx.enter_context(tc.tile_pool(name="psum", bufs=5, space="PSUM"))

        ident = singles.tile([128, 128], FP16)
        make_identity(nc, ident)

        # b1 [1, H] -> [128, H//128] with [p, hi] = b1[hi*128 + p]
        b1t = singles.tile([128, H // 128], FP)
        nc.scalar.dma_start(out=b1t, in_=b1.rearrange("o (h p) -> (o p) h", p=128))

        # b2 [1, D] broadcast across partitions -> [128, D]
        b2b = singles.tile([128, D], FP)
        b2_bcast = bass.AP(tensor=b2.tensor, offset=b2.offset,
                           ap=[[0, 128]] + [list(b2.ap[-1])])
        nc.scalar.dma_start(out=b2b, in_=b2_bcast)

        # w1s host layout [ht, p, kj, i, h'] -> per-ht loads are contiguous
        w1sv = w1s.rearrange("t (p r) -> t p r", p=128)

        # HAM warm-up with real matmuls
        wtile = singles.tile([128, 128], FP16)
        nc.vector.memset(wtile, 0.0)
        for i in range(56):
            pt = psum.tile([128, 128], FP, tag="psB", bufs=3, name=f"wu{i}")
            nc.tensor.matmul(pt, lhsT=wtile, rhs=wtile, start=True, stop=True)

        # x transposes, hybrid strategy:
        #  - half 1 (c4 0,1): PE transpose-mode (fast, needed immediately)
        #  - half 2 (c4 2,3): XBAR DMA-transpose (slow ~45GB/s, but issued at
        #    t~0 so its latency hides entirely under FC1 of half 1, costing
        #    zero PE/DVE work)
        xT = [[xt_pool.tile([128, 512], FP16, tag=f"xt{k}_{c4}",
                            name=f"xT{k}_{c4}")
               for c4 in range(2)] for k in range(N_KI)]
        xTc = [xt_pool.tile([128, N_KI, 512], FP16, tag=f"xtc{c4}",
                            name=f"xTc{c4}") for c4 in (2, 3)]
        for k in range(N_KI):
            xT[k].extend([xTc[0][:, k, :], xTc[1][:, k, :]])

        def emit_xpose(c4):
            if c4 >= 2:
                nc.sync.dma_start_transpose(
                    out=xTc[c4 - 2], in_=x[c4 * 512:(c4 + 1) * 512, :])
                return
            xs = xload.tile([128, 4, D], FP16, tag="xload", name=f"xs{c4}")
            nc.sync.dma_start(
                out=xs,
                in_=x[c4 * 512:(c4 + 1) * 512, :].rearrange(
                    "(r p) d -> p r d", p=128))
            for col in range(4):
                ti = c4 * 4 + col
                for k in range(N_KI):
                    pt = psum.tile([128, 128], FP16, tag="psB", bufs=3,
                                   name=f"psx{ti}_{k}")
                    nc.tensor.transpose(
                        out=pt,
                        in_=xs[:, col, k * 128:(k + 1) * 128],
                        identity=ident)
                    nc.vector.tensor_copy(
                        xT[k][c4][:, col * 128:(col + 1) * 128], pt)

        def emit_sides(hf):
            # transposes + A-side Strassen operands for half hf
            c4a, c4b = 2 * hf, 2 * hf + 1
            if hf == 0:
                emit_xpose(c4a)
            if hf == 0:
                # dependency-free PE filler while the second x chunk's DMA
                # lands (the c4b transposes would otherwise head-block the
                # PE queue)
                for i in range(48):
                    pt = psum.tile([128, 128], FP, tag="psB", bufs=3,
                                   name=f"wf{i}")
                    nc.tensor.matmul(pt, lhsT=wtile, rhs=wtile,
                                     start=True, stop=True)
            if hf == 0:
                emit_xpose(c4b)
            s = [[s_pool.tile([128, 512], FP16, tag=f"s{si}_{hf}_{kj}",
                              name=f"s{si}_{hf}_{kj}") for kj in range(N_KJ)]
                 for si in (1, 2, 3, 4)]
            s1, s2, s3, s4 = s
            # s1 first: products needing it (M5) run before M6/M7/M3
            for kj in range(N_KJ):
                nc.vector.tensor_add(s1[kj], xT[kj][c4b], xT[4 + kj][c4b])
            for kj in range(N_KJ):
                nc.vector.tensor_sub(s2[kj], s1[kj], xT[kj][c4a])
            for kj in range(N_KJ):
                nc.vector.tensor_sub(s3[kj], xT[kj][c4a], xT[kj][c4b])
            for kj in range(N_KJ):
                nc.vector.tensor_sub(s4[kj], xT[4 + kj][c4a], s2[kj])
            return s1, s2, s3, s4

        wp_cache = {}

        def wpt(hf, ht):
            k = (hf, ht)
            if k not in wp_cache:
                wp = w1s_pool.tile([128, N_KJ, 7, 128], FP16, tag="w1s",
                                   name=f"wp{hf}_{ht}")
                nc.scalar.dma_start(out=wp, in_=w1sv[ht])
                wp_cache[k] = wp
            return wp_cache[k]

        wpt(0, 0)   # first weight chunk ahead of the x stream on scalar
        sides = emit_sides(0)
        emit_xpose(2)
        emit_xpose(3)
        for hf in range(2):
            c4a, c4b = 2 * hf, 2 * hf + 1
            s1, s2, s3, s4 = sides

            # rhs tiles per product (index 1..7), per kj
            rhs_of = {
                1: [xT[kj][c4a] for kj in range(N_KJ)],
                2: [xT[4 + kj][c4a] for kj in range(N_KJ)],
                3: s4,
                4: [xT[4 + kj][c4b] for kj in range(N_KJ)],
                5: s1,
                6: s2,
                7: s3,
            }

            yth = [yt_pool.tile([128, TH], FP16, tag=f"yth{ht}",
                                name=f"yth{hf}_{ht}") for ht in range(N_HK)]

            w2_cache = {}

            def w2t(dc, hg, key):
                k = (dc, hg, key)
                if k not in w2_cache:
                    wt = w2_pool.tile([128, 4, 512], FP16, tag="w2",
                                      name=f"w2t{hf}_{dc}_{hg}_{key}")
                    nc.scalar.dma_start(
                        out=wt,
                        in_=w2[hg * 512:(hg + 1) * 512,
                               dc * 512:(dc + 1) * 512].rearrange(
                                   "(r p) d -> p r d", p=128))
                    w2_cache[k] = wt
                return w2_cache[k]

            for ht in range(N_HT):
                if ht == 11:
                    w2t(0, 0, f"{hf}_0")   # prefetch first sweep chunk
                if ht == 14:
                    w2t(0, 1, f"{hf}_0")
                wp = wpt(hf, ht)

                def product(i_prod, nm):
                    mt = psum.tile([128, 512], FP, tag="psA",
                                   name=f"m{nm}_{hf}_{ht}")
                    for kj in range(N_KJ):
                        nc.tensor.matmul(
                            mt,
                            lhsT=wp[:, kj, i_prod - 1, :],
                            rhs=rhs_of[i_prod][kj],
                            start=(kj == 0), stop=(kj == N_KJ - 1))
                    return mt

                # product order chosen so the PSUM ring slots of the first
                # allocations free early (M1 via the copy, M6/M7 via u2/u3):
                # the next group's matmuls then never wait on this group's
                # DVE assembly tail
                m1 = product(1, "1")
                m6 = product(6, "6")
                m7 = product(7, "7")

                # M1 is read twice; copy to SBUF on ScalarE so every DVE
                # tensor_tensor touches at most one PSUM bank
                m1c = us_pool.tile([128, 512], FP, tag="us", name=f"m1c{hf}_{ht}")
                nc.scalar.activation(out=m1c, in_=m1,
                                     func=mybir.ActivationFunctionType.Copy,
                                     scale=1.0)
                u2 = us_pool.tile([128, 512], FP, tag="us", name=f"u2_{hf}_{ht}")
                u3 = us_pool.tile([128, 512], FP, tag="us", name=f"u3_{hf}_{ht}")
                nc.vector.tensor_add(u2, m1c, m6)
                nc.vector.tensor_add(u3, u2, m7)

                m2 = product(2, "2")
                c11 = cs_pool.tile([128, 512], FP16, tag="cs", name=f"c11_{hf}_{ht}")
                nc.vector.tensor_add(c11, m1c, m2)
                m4 = product(4, "4")
                c21 = cs_pool.tile([128, 512], FP16, tag="cs", name=f"c21_{hf}_{ht}")
                nc.vector.tensor_sub(c21, u3, m4)
                m5 = product(5, "5")
                u4 = us_pool.tile([128, 512], FP, tag="us", name=f"u4_{hf}_{ht}")
                nc.vector.tensor_add(u4, u2, m5)
                c22 = cs_pool.tile([128, 512], FP16, tag="cs", name=f"c22_{hf}_{ht}")
                nc.vector.tensor_add(c22, u3, m5)
                m3 = product(3, "3")
                c12 = cs_pool.tile([128, 512], FP16, tag="cs", name=f"c12_{hf}_{ht}")
                nc.vector.tensor_add(c12, u4, m3)

                # bias + relu -> yT half tiles
                nc.scalar.activation(out=yth[ht][:, 0:512], in_=c11,
                                     func=RELU, bias=b1t[:, ht:ht + 1], scale=1.0)
                nc.scalar.activation(out=yth[ht][:, 512:1024], in_=c21,
                                     func=RELU, bias=b1t[:, ht:ht + 1], scale=1.0)
                nc.scalar.activation(out=yth[16 + ht][:, 0:512], in_=c12,
                                     func=RELU,
                                     bias=b1t[:, 16 + ht:17 + ht], scale=1.0)
                nc.scalar.activation(out=yth[16 + ht][:, 512:1024], in_=c22,
                                     func=RELU,
                                     bias=b1t[:, 16 + ht:17 + ht], scale=1.0)

            if hf == 0:
                sides = emit_sides(1)

            # ---- FC2 for this half: full contraction in PSUM ----
            # one sweep per 512-wide output chunk; 8 concurrent PSUM banks.
            # The very last sweep is split into two 4-til half-sweeps so the
            # final stores overlap the remaining matmuls instead of draining
            # ~1MB after the PE goes idle.
            def sweep(dc, tils, key):
                pts = {}
                for j, til in enumerate(tils):
                    tag = "psA" if j < 5 else "psB"
                    kw = dict(bufs=3) if j >= 5 else {}
                    pts[til] = psum.tile([128, 512], FP, tag=tag,
                                         name=f"psfc2_{key}_{til}", **kw)
                for hg in range(N_HK // 4):
                    wt = w2t(dc, hg, key)
                    for r in range(4):
                        hk = hg * 4 + r
                        for til in tils:
                            nc.tensor.matmul(
                                pts[til],
                                lhsT=yth[hk][:, til * 128:(til + 1) * 128],
                                rhs=wt[:, r, :],
                                start=(hk == 0), stop=(hk == N_HK - 1))
                for til in tils:
                    os = os_pool.tile([128, 512], FP16, tag="os",
                                      name=f"os_{key}_{til}")
                    if B2_IS_ZERO and til % 2 == 0:
                        # bias is identically zero: plain psum->sbuf copy can
                        # run on ScalarE, halving the serial drain at the end
                        # of each sweep (DVE handles the other half)
                        nc.scalar.activation(
                            out=os, in_=pts[til],
                            func=mybir.ActivationFunctionType.Copy, scale=1.0)
                    else:
                        nc.vector.tensor_add(os, pts[til],
                                             b2b[:, dc * 512:(dc + 1) * 512])
                    ti_abs = hf * N_TIL + til
                    ring = nc.sync if til % 2 == 0 else nc.scalar
                    ring.dma_start(
                        out=out[ti_abs * 128:(ti_abs + 1) * 128,
                                dc * 512:(dc + 1) * 512],
                        in_=os)

            sweep(0, list(range(N_TIL)), f"{hf}_0")
            sweep(1, list(range(N_TIL)), f"{hf}_1")


def build_module(b2_zero):
    global B2_IS_ZERO
    B2_IS_ZERO = b2_zero
    nc = bacc.Bacc("TRN2", target_bir_lowering=False, debug=False)
    x = nc.dram_tensor("x", [T, D], FP16, kind="ExternalInput").ap()
    w1s = nc.dram_tensor("w1s", [16, 128 * 4 * 7 * 128], FP16,
                         kind="ExternalInput").ap()
    b1 = nc.dram_tensor("fc1_b", [1, H], FP, kind="ExternalInput").ap()
    w2 = nc.dram_tensor("fc2_w", [H, D], FP16, kind="ExternalInput").ap()
    b2 = nc.dram_tensor("fc2_b", [1, D], FP, kind="ExternalInput").ap()
    out = nc.dram_tensor("out", [T, D], FP16, kind="ExternalOutput").ap()
    with tile.TileContext(nc) as tc:
        _emit_kernel(tc, out, x, w1s, b1, w2, b2)
    nc.compile()
    return nc


_CACHED = None


def _host_w1s(w1_f32):
    """Host-side Strassen-Winograd B-operands: [B11,B21,B22,T4,T1,T2,T3]."""
    b11 = w1_f32[0:512, 0:2048]
    b12 = w1_f32[0:512, 2048:4096]
    b21 = w1_f32[512:1024, 0:2048]
    b22 = w1_f32[512:1024, 2048:4096]
    t1 = b12 - b11
    t2 = b22 - t1
    t3 = b22 - b12
    t4 = t2 - b21
    w = np.stack([b11, b21, b22, t4, t1, t2, t3], axis=1)  # [512, 7, 2048]
    # -> [ht, p, kj, i, h'] so each per-ht slice is one contiguous DMA
    w5 = w.reshape(4, 128, 7, 16, 128).transpose(3, 1, 0, 2, 4)
    return np.ascontiguousarray(
        w5.reshape(16, 128 * 4 * 7 * 128).astype(np.float16))


def kernel(x, fc1_w, fc1_b, fc2_w, fc2_b, _trace=False, _trace_cores=None):
    b2_zero = bool(np.all(np.asarray(fc2_b) == 0.0))
    global _CACHED
    if _CACHED is None or _CACHED[0] != b2_zero:
        _CACHED = (b2_zero, build_module(b2_zero))
    nc = _CACHED[1]

    x = np.ascontiguousarray(np.asarray(x, dtype=np.float32).astype(np.float16))
    fc1_w = np.asarray(fc1_w, dtype=np.float32)
    fc1_b = np.ascontiguousarray(np.asarray(fc1_b, dtype=np.float32))
    fc2_w = np.ascontiguousarray(
        np.asarray(fc2_w, dtype=np.float32).astype(np.float16))
    fc2_b = np.ascontiguousarray(np.asarray(fc2_b, dtype=np.float32))

    in_maps = [
        {
            "x": x[e],
            "w1s": _host_w1s(fc1_w[e]),
            "fc1_b": fc1_b[e],
            "fc2_w": fc2_w[e],
            "fc2_b": fc2_b[e],
        }
        for e in range(E)
    ]
    kw = {}
    if _trace:
        kw = dict(trace=True,
                  trace_cores=_trace_cores if _trace_cores is not None else [0])
    res = run_bass_kernel_spmd(nc, in_maps, core_ids=list(range(NCORES)), **kw)
    out = np.stack([res.results[e]["out"].astype(np.float32)
                    for e in range(E)], axis=0)
    if _trace:
        return out, res
    return out


# revision 16
# speedup vs baseline: 1.2118x; 1.0172x over previous
"""Expert-parallel batched-expert FFN kernel for Trainium2 — Strassen FC1.

Reference computation (per expert e):
    y = relu(x[e] @ fc1_w[e] + fc1_b[e]) @ fc2_w[e] + fc2_b[e]

Sharding: E=8 experts, one expert per core (expert parallel, no collectives).

Per-core algorithm (T=2048 tokens, D=1024, H=4096), fp16 operands:
  - Tokens are processed in two halves of 1024.  Within a half, FC1 is
    computed with one level of Strassen-Winograd: A = x-half [1024, 1024]
    split into [512, 512] blocks, B = w1 [1024, 4096] into [512, 2048]
    blocks.  The 7 B-side operands (B11, B21, B22, T4, T1, T2, T3) are
    precomputed on the host and streamed; the 4 A-side operands S1..S4 are
    built on the DVE from the transposed x tiles; the 7 products run on the
    PE (7/8 of the classic MAC count); the C-quadrant assembly (7 adds) runs
    on the DVE out of PSUM, then ScalarE applies bias+relu producing the
    fp16 yT half [4096, 1024] resident in SBUF.
  - FC2 for a half accumulates its full 4096-deep contraction in PSUM
    (no partial-sum traffic): two sweeps (one per 512-wide output column
    chunk) of 8 concurrent [128, 512] PSUM banks; w2 columns stream per
    sweep.  Output tiles get bias via one DVE add (or a ScalarE copy when
    the bias is identically zero) and store as fp16, split across the two
    HW DMA rings to shorten the final drain.
  - x transposes are hybrid: half 1 on the PE (fast, ramp-critical), half 2
    via the XBAR DMA-transpose issued at t~0 whose ~45GB/s latency hides
    fully under half 1's compute, costing zero PE/DVE work.
  - Warm-up uses real dependency-free matmuls (HAM ignores transposes);
    PSUM ring slots and Winograd product order are arranged so no group's
    matmuls ever wait on the previous group's DVE assembly tail.
"""

from contextlib import ExitStack

import numpy as np

import concourse.bass as bass
import concourse.bacc as bacc
import concourse.mybir as mybir
import concourse.tile as tile
from concourse.bass_utils import run_bass_kernel_spmd
from concourse.masks import make_identity

E, T, D, H = 8, 2048, 1024, 4096
NCORES = 8
TH = T // 2                    # tokens per half
FP = mybir.dt.float32
FP16 = mybir.dt.float16
RELU = mybir.ActivationFunctionType.Relu

N_KI = D // 128                # 8  k-tiles of x
N_KJ = 4                       # k-tiles per Strassen d-block (512)
N_HT = 16                      # h-tiles per Strassen h-block (2048)
N_C4 = T // 512                # 4  512-token chunks
N_HK = H // 128                # 32 h k-tiles for FC2
N_TIL = TH // 128              # 8  token tiles per half
N_DC = D // 512                # 2


B2_IS_ZERO = False


def _emit_kernel(tc, out, x, w1s, b1, w2, b2):
    nc = tc.nc
    with ExitStack() as ctx:
        singles = ctx.enter_context(tc.tile_pool(name="singles", bufs=1))
        xload = ctx.enter_context(tc.tile_pool(name="xload", bufs=2))
        xt_pool = ctx.enter_context(tc.tile_pool(name="xt", bufs=1))
        s_pool = ctx.enter_context(tc.tile_pool(name="spool", bufs=1))
        yt_pool = ctx.enter_context(tc.tile_pool(name="yt", bufs=1))
        w1s_pool = ctx.enter_context(tc.tile_pool(name="w1s", bufs=3))
        w2_pool = ctx.enter_context(tc.tile_pool(name="w2", bufs=3))
        us_pool = ctx.enter_context(tc.tile_pool(name="us", bufs=4))
        cs_pool = ctx.enter_context(tc.tile_pool(name="cs", bufs=5))
        os_pool = ctx.enter_context(tc.tile_pool(name="os", bufs=3))
        psum = ctx.enter_context(tc.tile_pool(name="psum", bufs=5, space="PSUM"))

        ident = singles.tile([128, 128], FP16)
        make_identity(nc, ident)

        # b1 [1, H] -> [128, H//128] with [p, hi] = b1[hi*128 + p]
        b1t = singles.tile([128, H // 128], FP)
        nc.scalar.dma_start(out=b1t, in_=b1.rearrange("o (h p) -> (o p) h", p=128))

        # b2 [1, D] broadcast across partitions -> [128, D]
        b2b = singles.tile([128, D], FP)
        b2_bcast = bass.AP(tensor=b2.tensor, offset=b2.offset,
                           ap=[[0, 128]] + [list(b2.ap[-1])])
        nc.scalar.dma_start(out=b2b, in_=b2_bcast)

        # w1s host layout [ht, p, kj, i, h'] -> per-ht loads are contiguous
        w1sv = w1s.rearrange("t (p r) -> t p r", p=128)

        # HAM warm-up with real matmuls
        wtile = singles.tile([128, 128], FP16)
        nc.vector.memset(wtile, 0.0)
        for i in range(56):
            pt = psum.tile([128, 128], FP, tag="psB", bufs=3, name=f"wu{i}")
            nc.tensor.matmul(pt, lhsT=wtile, rhs=wtile, start=True, stop=True)

        # x transposes, hybrid strategy:
        #  - half 1 (c4 0,1): PE transpose-mode (fast, needed immediately)
        #  - half 2 (c4 2,3): XBAR DMA-transpose (slow ~45GB/s, but issued at
        #    t~0 so its latency hides entirely under FC1 of half 1, costing
        #    zero PE/DVE work)
        xT = [[xt_pool.tile([128, 512], FP16, tag=f"xt{k}_{c4}",
                            name=f"xT{k}_{c4}")
               for c4 in range(2)] for k in range(N_KI)]
        xTc = [xt_pool.tile([128, N_KI, 512], FP16, tag=f"xtc{c4}",
                            name=f"xTc{c4}") for c4 in (2, 3)]
        for k in range(N_KI):
            xT[k].extend([xTc[0][:, k, :], xTc[1][:, k, :]])

        def emit_xpose(c4):
            if c4 >= 2:
                nc.sync.dma_start_transpose(
                    out=xTc[c4 - 2], in_=x[c4 * 512:(c4 + 1) * 512, :])
                return
            xs = xload.tile([128, 4, D], FP16, tag="xload", name=f"xs{c4}")
            nc.sync.dma_start(
                out=xs,
                in_=x[c4 * 512:(c4 + 1) * 512, :].rearrange(
                    "(r p) d -> p r d", p=128))
            for col in range(4):
                ti = c4 * 4 + col
                for k in range(N_KI):
                    pt = psum.tile([128, 128], FP16, tag="psB", bufs=3,
                                   name=f"psx{ti}_{k}")
                    nc.tensor.transpose(
                        out=pt,
                        in_=xs[:, col, k * 128:(k + 1) * 128],
                        identity=ident)
                    nc.vector.tensor_copy(
                        xT[k][c4][:, col * 128:(col + 1) * 128], pt)

        def emit_sides(hf):
            # transposes + A-side Strassen operands for half hf
            c4a, c4b = 2 * hf, 2 * hf + 1
            if hf == 0:
                emit_xpose(c4a)
            if hf == 0:
                # dependency-free PE filler while the second x chunk's DMA
                # lands (the c4b transposes would otherwise head-block the
                # PE queue)
                for i in range(48):
                    pt = psum.tile([128, 128], FP, tag="psB", bufs=3,
                                   name=f"wf{i}")
                    nc.tensor.matmul(pt, lhsT=wtile, rhs=wtile,
                                     start=True, stop=True)
            if hf == 0:
                emit_xpose(c4b)
            s = [[s_pool.tile([128, 512], FP16, tag=f"s{si}_{hf}_{kj}",
                              name=f"s{si}_{hf}_{kj}") for kj in range(N_KJ)]
                 for si in (1, 2, 3, 4)]
            s1, s2, s3, s4 = s
            # s1 first: products needing it (M5) run before M6/M7/M3
            for kj in range(N_KJ):
                nc.vector.tensor_add(s1[kj], xT[kj][c4b], xT[4 + kj][c4b])
            for kj in range(N_KJ):
                nc.vector.tensor_sub(s2[kj], s1[kj], xT[kj][c4a])
            for kj in range(N_KJ):
                nc.vector.tensor_sub(s3[kj], xT[kj][c4a], xT[kj][c4b])
            for kj in range(N_KJ):
                nc.vector.tensor_sub(s4[kj], xT[4 + kj][c4a], s2[kj])
            return s1, s2, s3, s4

        wp_cache = {}

        def wpt(hf, ht):
            k = (hf, ht)
            if k not in wp_cache:
                wp = w1s_pool.tile([128, N_KJ, 7, 128], FP16, tag="w1s",
                                   name=f"wp{hf}_{ht}")
                nc.scalar.dma_start(out=wp, in_=w1sv[ht])
                wp_cache[k] = wp
            return wp_cache[k]

        wpt(0, 0)   # first weight chunk ahead of the x stream on scalar
        sides = emit_sides(0)
        emit_xpose(2)
        emit_xpose(3)
        for hf in range(2):
            c4a, c4b = 2 * hf, 2 * hf + 1
            s1, s2, s3, s4 = sides

            # rhs tiles per product (index 1..7), per kj
            rhs_of = {
                1: [xT[kj][c4a] for kj in range(N_KJ)],
                2: [xT[4 + kj][c4a] for kj in range(N_KJ)],
                3: s4,
                4: [xT[4 + kj][c4b] for kj in range(N_KJ)],
                5: s1,
                6: s2,
                7: s3,
            }

            yth = [yt_pool.tile([128, TH], FP16, tag=f"yth{ht}",
                                name=f"yth{hf}_{ht}") for ht in range(N_HK)]

            w2_cache = {}

            def w2t(dc, hg, key):
                k = (dc, hg, key)
                if k not in w2_cache:
                    wt = w2_pool.tile([128, 4, 512], FP16, tag="w2",
                                      name=f"w2t{hf}_{dc}_{hg}_{key}")
                    nc.scalar.dma_start(
                        out=wt,
                        in_=w2[hg * 512:(hg + 1) * 512,
                               dc * 512:(dc + 1) * 512].rearrange(
                                   "(r p) d -> p r d", p=128))
                    w2_cache[k] = wt
                return w2_cache[k]

            for ht in range(N_HT):
                if ht == 11:
                    w2t(0, 0, f"{hf}_0")   # prefetch first sweep chunk
                if ht == 14:
                    w2t(0, 1, f"{hf}_0")
                wp = wpt(hf, ht)

                def product(i_prod, nm):
                    mt = psum.tile([128, 512], FP, tag="psA",
                                   name=f"m{nm}_{hf}_{ht}")
                    for kj in range(N_KJ):
                        nc.tensor.matmul(
                            mt,
                            lhsT=wp[:, kj, i_prod - 1, :],
                            rhs=rhs_of[i_prod][kj],
                            start=(kj == 0), stop=(kj == N_KJ - 1))
                    return mt

                # product order chosen so the PSUM ring slots of the first
                # allocations free early (M1 via the copy, M6/M7 via u2/u3):
                # the next group's matmuls then never wait on this group's
                # DVE assembly tail
                m1 = product(1, "1")
                m6 = product(6, "6")
                m7 = product(7, "7")

                # M1 is read twice; copy to SBUF on ScalarE so every DVE
                # tensor_tensor touches at most one PSUM bank
                m1c = us_pool.tile([128, 512], FP, tag="us", name=f"m1c{hf}_{ht}")
                nc.scalar.activation(out=m1c, in_=m1,
                                     func=mybir.ActivationFunctionType.Copy,
                                     scale=1.0)
                u2 = us_pool.tile([128, 512], FP, tag="us", name=f"u2_{hf}_{ht}")
                u3 = us_pool.tile([128, 512], FP, tag="us", name=f"u3_{hf}_{ht}")
                nc.vector.tensor_add(u2, m1c, m6)
                nc.vector.tensor_add(u3, u2, m7)

                m2 = product(2, "2")
                c11 = cs_pool.tile([128, 512], FP16, tag="cs", name=f"c11_{hf}_{ht}")
                nc.vector.tensor_add(c11, m1c, m2)
                m4 = product(4, "4")
                c21 = cs_pool.tile([128, 512], FP16, tag="cs", name=f"c21_{hf}_{ht}")
                nc.vector.tensor_sub(c21, u3, m4)
                m5 = product(5, "5")
                u4 = us_pool.tile([128, 512], FP, tag="us", name=f"u4_{hf}_{ht}")
                nc.vector.tensor_add(u4, u2, m5)
                c22 = cs_pool.tile([128, 512], FP16, tag="cs", name=f"c22_{hf}_{ht}")
                nc.vector.tensor_add(c22, u3, m5)
                m3 = product(3, "3")
                c12 = cs_pool.tile([128, 512], FP16, tag="cs", name=f"c12_{hf}_{ht}")
                nc.vector.tensor_add(c12, u4, m3)

                # bias + relu -> yT half tiles
                nc.scalar.activation(out=yth[ht][:, 0:512], in_=c11,
                                     func=RELU, bias=b1t[:, ht:ht + 1], scale=1.0)
                nc.scalar.activation(out=yth[ht][:, 512:1024], in_=c21,
                                     func=RELU, bias=b1t[:, ht:ht + 1], scale=1.0)
                nc.scalar.activation(out=yth[16 + ht][:, 0:512], in_=c12,
                                     func=RELU,
                                     bias=b1t[:, 16 + ht:17 + ht], scale=1.0)
                nc.scalar.activation(out=yth[16 + ht][:, 512:1024], in_=c22,
                                     func=RELU,
                                     bias=b1t[:, 16 + ht:17 + ht], scale=1.0)

            if hf == 0:
                sides = emit_sides(1)

            # ---- FC2 for this half: full contraction in PSUM ----
            # one sweep per 512-wide output chunk; 8 concurrent PSUM banks.
            # The very last sweep is split into two 4-til half-sweeps so the
            # final stores overlap the remaining matmuls instead of draining
            # ~1MB after the PE goes idle.
            def sweep(dc, tils, key):
                pts = {}
                for j, til in enumerate(tils):
                    tag = "psA" if j < 5 else "psB"
                    kw = dict(bufs=3) if j >= 5 else {}
                    pts[til] = psum.tile([128, 512], FP, tag=tag,
                                         name=f"psfc2_{key}_{til}", **kw)
                for hg in range(N_HK // 4):
                    wt = w2t(dc, hg, key)
                    for r in range(4):
                        hk = hg * 4 + r
                        for til in tils:
                            nc.tensor.matmul(
                                pts[til],
                                lhsT=yth[hk][:, til * 128:(til + 1) * 128],
                                rhs=wt[:, r, :],
                                start=(hk == 0), stop=(hk == N_HK - 1))
                for til in tils:
                    os = os_pool.tile([128, 512], FP16, tag="os",
                                      name=f"os_{key}_{til}")
                    if B2_IS_ZERO and til % 2 == 0:
                        # bias is identically zero: plain psum->sbuf copy can
                        # run on ScalarE, halving the serial drain at the end
                        # of each sweep (DVE handles the other half)
                        nc.scalar.activation(
                            out=os, in_=pts[til],
                            func=mybir.ActivationFunctionType.Copy, scale=1.0)
                    else:
                        nc.vector.tensor_add(os, pts[til],
                                             b2b[:, dc * 512:(dc + 1) * 512])
                    ti_abs = hf * N_TIL + til
                    ring = nc.sync if til % 2 == 0 else nc.scalar
                    ring.dma_start(
                        out=out[ti_abs * 128:(ti_abs + 1) * 128,
                                dc * 512:(dc + 1) * 512],
                        in_=os)

            sweep(0, list(range(N_TIL)), f"{hf}_0")
            sweep(1, list(range(N_TIL)), f"{hf}_1")


def build_module(b2_zero):
    global B2_IS_ZERO
    B2_IS_ZERO = b2_zero
    nc = bacc.Bacc("TRN2", target_bir_lowering=False, debug=False)
    x = nc.dram_tensor("x", [T, D], FP16, kind="ExternalInput").ap()
    w1s = nc.dram_tensor("w1s", [16, 128 * 4 * 7 * 128], FP16,
                         kind="ExternalInput").ap()
    b1 = nc.dram_tensor("fc1_b", [1, H], FP, kind="ExternalInput").ap()
    w2 = nc.dram_tensor("fc2_w", [H, D], FP16, kind="ExternalInput").ap()
    b2 = nc.dram_tensor("fc2_b", [1, D], FP, kind="ExternalInput").ap()
    out = nc.dram_tensor("out", [T, D], FP16, kind="ExternalOutput").ap()
    with tile.TileContext(nc) as tc:
        _emit_kernel(tc, out, x, w1s, b1, w2, b2)
    nc.compile()
    return nc


_CACHED = None


def _host_w1s(w1_f32):
    """Host-side Strassen-Winograd B-operands: [B11,B21,B22,T4,T1,T2,T3]."""
    b11 = w1_f32[0:512, 0:2048]
    b12 = w1_f32[0:512, 2048:4096]
    b21 = w1_f32[512:1024, 0:2048]
    b22 = w1_f32[512:1024, 2048:4096]
    t1 = b12 - b11
    t2 = b22 - t1
    t3 = b22 - b12
    t4 = t2 - b21
    w = np.stack([b11, b21, b22, t4, t1, t2, t3], axis=1)  # [512, 7, 2048]
    # -> [ht, p, kj, i, h'] so each per-ht slice is one contiguous DMA
    w5 = w.reshape(4, 128, 7, 16, 128).transpose(3, 1, 0, 2, 4)
    return np.ascontiguousarray(
        w5.reshape(16, 128 * 4 * 7 * 128).astype(np.float16))


def kernel(x, fc1_w, fc1_b, fc2_w, fc2_b, _trace=False, _trace_cores=None):
    b2_zero = bool(np.all(np.asarray(fc2_b) == 0.0))
    global _CACHED
    if _CACHED is None or _CACHED[0] != b2_zero:
        _CACHED = (b2_zero, build_module(b2_zero))
    nc = _CACHED[1]

    x = np.ascontiguousarray(np.asarray(x, dtype=np.float32).astype(np.float16))
    fc1_w = np.asarray(fc1_w, dtype=np.float32)
    fc1_b = np.ascontiguousarray(np.asarray(fc1_b, dtype=np.float32))
    fc2_w = np.ascontiguousarray(
        np.asarray(fc2_w, dtype=np.float32).astype(np.float16))
    fc2_b = np.ascontiguousarray(np.asarray(fc2_b, dtype=np.float32))

    in_maps = [
        {
            "x": x[e],
            "w1s": _host_w1s(fc1_w[e]),
            "fc1_b": fc1_b[e],
            "fc2_w": fc2_w[e],
            "fc2_b": fc2_b[e],
        }
        for e in range(E)
    ]
    kw = {}
    if _trace:
        kw = dict(trace=True,
                  trace_cores=_trace_cores if _trace_cores is not None else [0])
    res = run_bass_kernel_spmd(nc, in_maps, core_ids=list(range(NCORES)), **kw)
    out = np.stack([res.results[e]["out"].astype(np.float32)
                    for e in range(E)], axis=0)
    if _trace:
        return out, res
    return out


# revision 17
# speedup vs baseline: 1.2146x; 1.0023x over previous
"""Expert-parallel batched-expert FFN kernel for Trainium2 — Strassen FC1.

Reference computation (per expert e):
    y = relu(x[e] @ fc1_w[e] + fc1_b[e]) @ fc2_w[e] + fc2_b[e]

Sharding: E=8 experts, one expert per core (expert parallel, no collectives).

Per-core algorithm (T=2048 tokens, D=1024, H=4096), fp16 operands:
  - Tokens are processed in two halves of 1024.  Within a half, FC1 is
    computed with one level of Strassen-Winograd: A = x-half [1024, 1024]
    split into [512, 512] blocks, B = w1 [1024, 4096] into [512, 2048]
    blocks.  The 7 B-side operands (B11, B21, B22, T4, T1, T2, T3) are
    precomputed on the host and streamed; the 4 A-side operands S1..S4 are
    built on the DVE from the transposed x tiles; the 7 products run on the
    PE (7/8 of the classic MAC count); the C-quadrant assembly (7 adds) runs
    on the DVE out of PSUM, then ScalarE applies bias+relu producing the
    fp16 yT half [4096, 1024] resident in SBUF.
  - FC2 for a half accumulates its full 4096-deep contraction in PSUM
    (no partial-sum traffic): two sweeps (one per 512-wide output column
    chunk) of 8 concurrent [128, 512] PSUM banks; w2 columns stream per
    sweep.  Output tiles get bias via one DVE add (or a ScalarE copy when
    the bias is identically zero) and store as fp16, split across the two
    HW DMA rings to shorten the final drain.
  - x transposes are hybrid: half 1 on the PE (fast, ramp-critical), half 2
    via the XBAR DMA-transpose issued at t~0 whose ~45GB/s latency hides
    fully under half 1's compute, costing zero PE/DVE work.
  - Warm-up uses real dependency-free matmuls (HAM ignores transposes);
    PSUM ring slots and Winograd product order are arranged so no group's
    matmuls ever wait on the previous group's DVE assembly tail.
"""

from contextlib import ExitStack

import numpy as np

import concourse.bass as bass
import concourse.bacc as bacc
import concourse.mybir as mybir
import concourse.tile as tile
from concourse.bass_utils import run_bass_kernel_spmd
from concourse.masks import make_identity

E, T, D, H = 8, 2048, 1024, 4096
NCORES = 8
TH = T // 2                    # tokens per half
FP = mybir.dt.float32
FP16 = mybir.dt.float16
RELU = mybir.ActivationFunctionType.Relu

N_KI = D // 128                # 8  k-tiles of x
N_KJ = 4                       # k-tiles per Strassen d-block (512)
N_HT = 16                      # h-tiles per Strassen h-block (2048)
N_C4 = T // 512                # 4  512-token chunks
N_HK = H // 128                # 32 h k-tiles for FC2
N_TIL = TH // 128              # 8  token tiles per half
N_DC = D // 512                # 2


B2_IS_ZERO = False


def _emit_kernel(tc, out, x, w1s, b1, w2, b2):
    nc = tc.nc
    with ExitStack() as ctx:
        singles = ctx.enter_context(tc.tile_pool(name="singles", bufs=1))
        xload = ctx.enter_context(tc.tile_pool(name="xload", bufs=2))
        xt_pool = ctx.enter_context(tc.tile_pool(name="xt", bufs=1))
        s_pool = ctx.enter_context(tc.tile_pool(name="spool", bufs=1))
        yt_pool = ctx.enter_context(tc.tile_pool(name="yt", bufs=1))
        w1s_pool = ctx.enter_context(tc.tile_pool(name="w1s", bufs=3))
        w2_pool = ctx.enter_context(tc.tile_pool(name="w2", bufs=3))
        us_pool = ctx.enter_context(tc.tile_pool(name="us", bufs=4))
        cs_pool = ctx.enter_context(tc.tile_pool(name="cs", bufs=5))
        os_pool = ctx.enter_context(tc.tile_pool(name="os", bufs=8))
        psum = ctx.enter_context(tc.tile_pool(name="psum", bufs=5, space="PSUM"))

        ident = singles.tile([128, 128], FP16)
        make_identity(nc, ident)

        # b1 [1, H] -> [128, H//128] with [p, hi] = b1[hi*128 + p]
        b1t = singles.tile([128, H // 128], FP)
        nc.scalar.dma_start(out=b1t, in_=b1.rearrange("o (h p) -> (o p) h", p=128))

        # b2 [1, D] broadcast across partitions -> [128, D]
        b2b = singles.tile([128, D], FP)
        b2_bcast = bass.AP(tensor=b2.tensor, offset=b2.offset,
                           ap=[[0, 128]] + [list(b2.ap[-1])])
        nc.scalar.dma_start(out=b2b, in_=b2_bcast)

        # w1s host layout [ht, p, kj, i, h'] -> per-ht loads are contiguous
        w1sv = w1s.rearrange("t (p r) -> t p r", p=128)

        # HAM warm-up with real matmuls
        wtile = singles.tile([128, 128], FP16)
        nc.vector.memset(wtile, 0.0)
        for i in range(56):
            pt = psum.tile([128, 128], FP, tag="psB", bufs=3, name=f"wu{i}")
            nc.tensor.matmul(pt, lhsT=wtile, rhs=wtile, start=True, stop=True)

        # x transposes, hybrid strategy:
        #  - half 1 (c4 0,1): PE transpose-mode (fast, needed immediately)
        #  - half 2 (c4 2,3): XBAR DMA-transpose (slow ~45GB/s, but issued at
        #    t~0 so its latency hides entirely under FC1 of half 1, costing
        #    zero PE/DVE work)
        xT = [[xt_pool.tile([128, 512], FP16, tag=f"xt{k}_{c4}",
                            name=f"xT{k}_{c4}")
               for c4 in range(2)] for k in range(N_KI)]
        xTc = [xt_pool.tile([128, N_KI, 512], FP16, tag=f"xtc{c4}",
                            name=f"xTc{c4}") for c4 in (2, 3)]
        for k in range(N_KI):
            xT[k].extend([xTc[0][:, k, :], xTc[1][:, k, :]])

        def emit_xpose(c4):
            if c4 >= 2:
                nc.sync.dma_start_transpose(
                    out=xTc[c4 - 2], in_=x[c4 * 512:(c4 + 1) * 512, :])
                return
            xs = xload.tile([128, 4, D], FP16, tag="xload", name=f"xs{c4}")
            nc.sync.dma_start(
                out=xs,
                in_=x[c4 * 512:(c4 + 1) * 512, :].rearrange(
                    "(r p) d -> p r d", p=128))
            for col in range(4):
                ti = c4 * 4 + col
                for k in range(N_KI):
                    pt = psum.tile([128, 128], FP16, tag="psB", bufs=3,
                                   name=f"psx{ti}_{k}")
                    nc.tensor.transpose(
                        out=pt,
                        in_=xs[:, col, k * 128:(k + 1) * 128],
                        identity=ident)
                    nc.vector.tensor_copy(
                        xT[k][c4][:, col * 128:(col + 1) * 128], pt)

        def emit_sides(hf):
            # transposes + A-side Strassen operands for half hf
            c4a, c4b = 2 * hf, 2 * hf + 1
            if hf == 0:
                emit_xpose(c4a)
            if hf == 0:
                # dependency-free PE filler while the second x chunk's DMA
                # lands (the c4b transposes would otherwise head-block the
                # PE queue)
                for i in range(48):
                    pt = psum.tile([128, 128], FP, tag="psB", bufs=3,
                                   name=f"wf{i}")
                    nc.tensor.matmul(pt, lhsT=wtile, rhs=wtile,
                                     start=True, stop=True)
            if hf == 0:
                emit_xpose(c4b)
            s = [[s_pool.tile([128, 512], FP16, tag=f"s{si}_{kj}",
                              name=f"s{si}_{hf}_{kj}") for kj in range(N_KJ)]
                 for si in (1, 2, 3, 4)]
            s1, s2, s3, s4 = s
            # s1 first: products needing it (M5) run before M6/M7/M3
            for kj in range(N_KJ):
                nc.vector.tensor_add(s1[kj], xT[kj][c4b], xT[4 + kj][c4b])
            for kj in range(N_KJ):
                nc.vector.tensor_sub(s2[kj], s1[kj], xT[kj][c4a])
            for kj in range(N_KJ):
                nc.vector.tensor_sub(s3[kj], xT[kj][c4a], xT[kj][c4b])
            for kj in range(N_KJ):
                nc.vector.tensor_sub(s4[kj], xT[4 + kj][c4a], s2[kj])
            return s1, s2, s3, s4

        wp_cache = {}

        def wpt(hf, ht):
            k = (hf, ht)
            if k not in wp_cache:
                wp = w1s_pool.tile([128, N_KJ, 7, 128], FP16, tag="w1s",
                                   name=f"wp{hf}_{ht}")
                nc.scalar.dma_start(out=wp, in_=w1sv[ht])
                wp_cache[k] = wp
            return wp_cache[k]

        wpt(0, 0)   # first weight chunk ahead of the x stream on scalar
        sides = emit_sides(0)
        emit_xpose(2)
        emit_xpose(3)
        for hf in range(2):
            c4a, c4b = 2 * hf, 2 * hf + 1
            s1, s2, s3, s4 = sides

            # rhs tiles per product (index 1..7), per kj
            rhs_of = {
                1: [xT[kj][c4a] for kj in range(N_KJ)],
                2: [xT[4 + kj][c4a] for kj in range(N_KJ)],
                3: s4,
                4: [xT[4 + kj][c4b] for kj in range(N_KJ)],
                5: s1,
                6: s2,
                7: s3,
            }

            yth = [yt_pool.tile([128, TH], FP16, tag=f"yth{ht}",
                                name=f"yth{hf}_{ht}") for ht in range(N_HK)]

            w2_cache = {}

            def w2t(dc, hg, key):
                k = (dc, hg, key)
                if k not in w2_cache:
                    wt = w2_pool.tile([128, 4, 512], FP16, tag="w2",
                                      name=f"w2t{hf}_{dc}_{hg}_{key}")
                    nc.scalar.dma_start(
                        out=wt,
                        in_=w2[hg * 512:(hg + 1) * 512,
                               dc * 512:(dc + 1) * 512].rearrange(
                                   "(r p) d -> p r d", p=128))
                    w2_cache[k] = wt
                return w2_cache[k]

            for ht in range(N_HT):
                if ht == 11:
                    w2t(0, 0, f"{hf}_0")   # prefetch first sweep chunk
                if ht == 14:
                    w2t(0, 1, f"{hf}_0")
                wp = wpt(hf, ht)

                def product(i_prod, nm):
                    mt = psum.tile([128, 512], FP, tag="psA",
                                   name=f"m{nm}_{hf}_{ht}")
                    for kj in range(N_KJ):
                        nc.tensor.matmul(
                            mt,
                            lhsT=wp[:, kj, i_prod - 1, :],
                            rhs=rhs_of[i_prod][kj],
                            start=(kj == 0), stop=(kj == N_KJ - 1))
                    return mt

                # product order chosen so the PSUM ring slots of the first
                # allocations free early (M1 via the copy, M6/M7 via u2/u3):
                # the next group's matmuls then never wait on this group's
                # DVE assembly tail
                m1 = product(1, "1")
                m6 = product(6, "6")
                m7 = product(7, "7")

                # M1 is read twice; copy to SBUF on ScalarE so every DVE
                # tensor_tensor touches at most one PSUM bank
                m1c = us_pool.tile([128, 512], FP, tag="us", name=f"m1c{hf}_{ht}")
                nc.scalar.activation(out=m1c, in_=m1,
                                     func=mybir.ActivationFunctionType.Copy,
                                     scale=1.0)
                u2 = us_pool.tile([128, 512], FP, tag="us", name=f"u2_{hf}_{ht}")
                u3 = us_pool.tile([128, 512], FP, tag="us", name=f"u3_{hf}_{ht}")
                nc.vector.tensor_add(u2, m1c, m6)
                nc.vector.tensor_add(u3, u2, m7)

                m2 = product(2, "2")
                c11 = cs_pool.tile([128, 512], FP16, tag="cs", name=f"c11_{hf}_{ht}")
                nc.vector.tensor_add(c11, m1c, m2)
                m4 = product(4, "4")
                c21 = cs_pool.tile([128, 512], FP16, tag="cs", name=f"c21_{hf}_{ht}")
                nc.vector.tensor_sub(c21, u3, m4)
                m5 = product(5, "5")
                u4 = us_pool.tile([128, 512], FP, tag="us", name=f"u4_{hf}_{ht}")
                nc.vector.tensor_add(u4, u2, m5)
                c22 = cs_pool.tile([128, 512], FP16, tag="cs", name=f"c22_{hf}_{ht}")
                nc.vector.tensor_add(c22, u3, m5)
                m3 = product(3, "3")
                c12 = cs_pool.tile([128, 512], FP16, tag="cs", name=f"c12_{hf}_{ht}")
                nc.vector.tensor_add(c12, u4, m3)

                # bias + relu -> yT half tiles
                nc.scalar.activation(out=yth[ht][:, 0:512], in_=c11,
                                     func=RELU, bias=b1t[:, ht:ht + 1], scale=1.0)
                nc.scalar.activation(out=yth[ht][:, 512:1024], in_=c21,
                                     func=RELU, bias=b1t[:, ht:ht + 1], scale=1.0)
                nc.scalar.activation(out=yth[16 + ht][:, 0:512], in_=c12,
                                     func=RELU,
                                     bias=b1t[:, 16 + ht:17 + ht], scale=1.0)
                nc.scalar.activation(out=yth[16 + ht][:, 512:1024], in_=c22,
                                     func=RELU,
                                     bias=b1t[:, 16 + ht:17 + ht], scale=1.0)

            if hf == 0:
                sides = emit_sides(1)

            # ---- FC2 for this half: full contraction in PSUM ----
            # one sweep per 512-wide output chunk; 8 concurrent PSUM banks.
            # The very last sweep is split into two 4-til half-sweeps so the
            # final stores overlap the remaining matmuls instead of draining
            # ~1MB after the PE goes idle.
            def sweep(dc, tils, key):
                pts = {}
                for j, til in enumerate(tils):
                    tag = "psA" if j < 5 else "psB"
                    kw = dict(bufs=3) if j >= 5 else {}
                    pts[til] = psum.tile([128, 512], FP, tag=tag,
                                         name=f"psfc2_{key}_{til}", **kw)
                for hg in range(N_HK // 4):
                    wt = w2t(dc, hg, key)
                    for r in range(4):
                        hk = hg * 4 + r
                        for til in tils:
                            nc.tensor.matmul(
                                pts[til],
                                lhsT=yth[hk][:, til * 128:(til + 1) * 128],
                                rhs=wt[:, r, :],
                                start=(hk == 0), stop=(hk == N_HK - 1))
                for til in tils:
                    os = os_pool.tile([128, 512], FP16, tag="os",
                                      name=f"os_{key}_{til}")
                    if B2_IS_ZERO and til % 2 == 0:
                        # bias is identically zero: plain psum->sbuf copy can
                        # run on ScalarE, halving the serial drain at the end
                        # of each sweep (DVE handles the other half)
                        nc.scalar.activation(
                            out=os, in_=pts[til],
                            func=mybir.ActivationFunctionType.Copy, scale=1.0)
                    else:
                        nc.vector.tensor_add(os, pts[til],
                                             b2b[:, dc * 512:(dc + 1) * 512])
                    ti_abs = hf * N_TIL + til
                    ring = nc.sync if til % 2 == 0 else nc.scalar
                    ring.dma_start(
                        out=out[ti_abs * 128:(ti_abs + 1) * 128,
                                dc * 512:(dc + 1) * 512],
                        in_=os)

            sweep(0, list(range(N_TIL)), f"{hf}_0")
            sweep(1, list(range(N_TIL)), f"{hf}_1")


def build_module(b2_zero):
    global B2_IS_ZERO
    B2_IS_ZERO = b2_zero
    nc = bacc.Bacc("TRN2", target_bir_lowering=False, debug=False)
    x = nc.dram_tensor("x", [T, D], FP16, kind="ExternalInput").ap()
    w1s = nc.dram_tensor("w1s", [16, 128 * 4 * 7 * 128], FP16,
                         kind="ExternalInput").ap()
    b1 = nc.dram_tensor("fc1_b", [1, H], FP, kind="ExternalInput").ap()
    w2 = nc.dram_tensor("fc2_w", [H, D], FP16, kind="ExternalInput").ap()
    b2 = nc.dram_tensor("fc2_b", [1, D], FP, kind="ExternalInput").ap()
    out = nc.dram_tensor("out", [T, D], FP16, kind="ExternalOutput").ap()
    with tile.TileContext(nc) as tc:
        _emit_kernel(tc, out, x, w1s, b1, w2, b2)
    nc.compile()
    return nc


_CACHED = None


def _host_w1s(w1_f32):
    """Host-side Strassen-Winograd B-operands: [B11,B21,B22,T4,T1,T2,T3]."""
    b11 = w1_f32[0:512, 0:2048]
    b12 = w1_f32[0:512, 2048:4096]
    b21 = w1_f32[512:1024, 0:2048]
    b22 = w1_f32[512:1024, 2048:4096]
    t1 = b12 - b11
    t2 = b22 - t1
    t3 = b22 - b12
    t4 = t2 - b21
    w = np.stack([b11, b21, b22, t4, t1, t2, t3], axis=1)  # [512, 7, 2048]
    # -> [ht, p, kj, i, h'] so each per-ht slice is one contiguous DMA
    w5 = w.reshape(4, 128, 7, 16, 128).transpose(3, 1, 0, 2, 4)
    return np.ascontiguousarray(
        w5.reshape(16, 128 * 4 * 7 * 128).astype(np.float16))


def kernel(x, fc1_w, fc1_b, fc2_w, fc2_b, _trace=False, _trace_cores=None):
    b2_zero = bool(np.all(np.asarray(fc2_b) == 0.0))
    global _CACHED
    if _CACHED is None or _CACHED[0] != b2_zero:
        _CACHED = (b2_zero, build_module(b2_zero))
    nc = _CACHED[1]

    x = np.ascontiguousarray(np.asarray(x, dtype=np.float32).astype(np.float16))
    fc1_w = np.asarray(fc1_w, dtype=np.float32)
    fc1_b = np.ascontiguousarray(np.asarray(fc1_b, dtype=np.float32))
    fc2_w = np.ascontiguousarray(
        np.asarray(fc2_w, dtype=np.float32).astype(np.float16))
    fc2_b = np.ascontiguousarray(np.asarray(fc2_b, dtype=np.float32))

    in_maps = [
        {
            "x": x[e],
            "w1s": _host_w1s(fc1_w[e]),
            "fc1_b": fc1_b[e],
            "fc2_w": fc2_w[e],
            "fc2_b": fc2_b[e],
        }
        for e in range(E)
    ]
    kw = {}
    if _trace:
        kw = dict(trace=True,
                  trace_cores=_trace_cores if _trace_cores is not None else [0])
    res = run_bass_kernel_spmd(nc, in_maps, core_ids=list(range(NCORES)), **kw)
    out = np.stack([res.results[e]["out"].astype(np.float32)
                    for e in range(E)], axis=0)
    if _trace:
        return out, res
    return out


# revision 18
# speedup vs baseline: 1.2157x; 1.0010x over previous
"""Expert-parallel batched-expert FFN kernel for Trainium2 — Strassen FC1.

Reference computation (per expert e):
    y = relu(x[e] @ fc1_w[e] + fc1_b[e]) @ fc2_w[e] + fc2_b[e]

Sharding: E=8 experts, one expert per core (expert parallel, no collectives).

Per-core algorithm (T=2048 tokens, D=1024, H=4096), fp16 operands:
  - Tokens are processed in two halves of 1024.  Within a half, FC1 is
    computed with one level of Strassen-Winograd: A = x-half [1024, 1024]
    split into [512, 512] blocks, B = w1 [1024, 4096] into [512, 2048]
    blocks.  The 7 B-side operands (B11, B21, B22, T4, T1, T2, T3) are
    precomputed on the host and streamed; the 4 A-side operands S1..S4 are
    built on the DVE from the transposed x tiles; the 7 products run on the
    PE (7/8 of the classic MAC count); the C-quadrant assembly (7 adds) runs
    on the DVE out of PSUM, then ScalarE applies bias+relu producing the
    fp16 yT half [4096, 1024] resident in SBUF.
  - FC2 for a half accumulates its full 4096-deep contraction in PSUM
    (no partial-sum traffic): two sweeps (one per 512-wide output column
    chunk) of 8 concurrent [128, 512] PSUM banks; w2 columns stream per
    sweep.  Output tiles get bias via one DVE add (or a ScalarE copy when
    the bias is identically zero) and store as fp16, split across the two
    HW DMA rings to shorten the final drain.
  - x transposes are hybrid: half 1 on the PE (fast, ramp-critical), half 2
    via the XBAR DMA-transpose issued at t~0 whose ~45GB/s latency hides
    fully under half 1's compute, costing zero PE/DVE work.
  - Warm-up uses real dependency-free matmuls (HAM ignores transposes);
    PSUM ring slots and Winograd product order are arranged so no group's
    matmuls ever wait on the previous group's DVE assembly tail.
"""

from contextlib import ExitStack

import numpy as np

import concourse.bass as bass
import concourse.bacc as bacc
import concourse.mybir as mybir
import concourse.tile as tile
from concourse.bass_utils import run_bass_kernel_spmd
from concourse.masks import make_identity

E, T, D, H = 8, 2048, 1024, 4096
NCORES = 8
TH = T // 2                    # tokens per half
FP = mybir.dt.float32
FP16 = mybir.dt.float16
RELU = mybir.ActivationFunctionType.Relu

N_KI = D // 128                # 8  k-tiles of x
N_KJ = 4                       # k-tiles per Strassen d-block (512)
N_HT = 16                      # h-tiles per Strassen h-block (2048)
N_C4 = T // 512                # 4  512-token chunks
N_HK = H // 128                # 32 h k-tiles for FC2
N_TIL = TH // 128              # 8  token tiles per half
N_DC = D // 512                # 2


B2_IS_ZERO = False


def _emit_kernel(tc, out, x, w1s, b1, w2, b2):
    nc = tc.nc
    with ExitStack() as ctx:
        singles = ctx.enter_context(tc.tile_pool(name="singles", bufs=1))
        xload = ctx.enter_context(tc.tile_pool(name="xload", bufs=2))
        xt_pool = ctx.enter_context(tc.tile_pool(name="xt", bufs=1))
        s_pool = ctx.enter_context(tc.tile_pool(name="spool", bufs=1))
        yt_pool = ctx.enter_context(tc.tile_pool(name="yt", bufs=1))
        w1s_pool = ctx.enter_context(tc.tile_pool(name="w1s", bufs=3))
        w2_pool = ctx.enter_context(tc.tile_pool(name="w2", bufs=3))
        us_pool = ctx.enter_context(tc.tile_pool(name="us", bufs=4))
        cs_pool = ctx.enter_context(tc.tile_pool(name="cs", bufs=5))
        os_pool = ctx.enter_context(tc.tile_pool(name="os", bufs=8))
        psum = ctx.enter_context(tc.tile_pool(name="psum", bufs=5, space="PSUM"))

        ident = singles.tile([128, 128], FP16)
        make_identity(nc, ident)

        # b1 [1, H] -> [128, H//128] with [p, hi] = b1[hi*128 + p]
        b1t = singles.tile([128, H // 128], FP)
        nc.scalar.dma_start(out=b1t, in_=b1.rearrange("o (h p) -> (o p) h", p=128))

        # b2 [1, D] broadcast across partitions -> [128, D]
        b2b = singles.tile([128, D], FP)
        b2_bcast = bass.AP(tensor=b2.tensor, offset=b2.offset,
                           ap=[[0, 128]] + [list(b2.ap[-1])])
        nc.scalar.dma_start(out=b2b, in_=b2_bcast)

        # w1s host layout [ht, p, kj, i, h'] -> per-ht loads are contiguous
        w1sv = w1s.rearrange("t (p r) -> t p r", p=128)

        # HAM warm-up with real matmuls
        wtile = singles.tile([128, 128], FP16)
        nc.vector.memset(wtile, 0.0)
        for i in range(56):
            pt = psum.tile([128, 128], FP, tag="psB", bufs=3, name=f"wu{i}")
            nc.tensor.matmul(pt, lhsT=wtile, rhs=wtile, start=True, stop=True)

        # x transposes, hybrid strategy:
        #  - half 1 (c4 0,1): PE transpose-mode (fast, needed immediately)
        #  - half 2 (c4 2,3): XBAR DMA-transpose (slow ~45GB/s, but issued at
        #    t~0 so its latency hides entirely under FC1 of half 1, costing
        #    zero PE/DVE work)
        xT = [[xt_pool.tile([128, 512], FP16, tag=f"xt{k}_{c4}",
                            name=f"xT{k}_{c4}")
               for c4 in range(2)] for k in range(N_KI)]
        xTc = [xt_pool.tile([128, N_KI, 512], FP16, tag=f"xtc{c4}",
                            name=f"xTc{c4}") for c4 in (2, 3)]
        for k in range(N_KI):
            xT[k].extend([xTc[0][:, k, :], xTc[1][:, k, :]])

        def emit_xpose(c4):
            if c4 >= 2:
                nc.sync.dma_start_transpose(
                    out=xTc[c4 - 2], in_=x[c4 * 512:(c4 + 1) * 512, :])
                return
            xs = xload.tile([128, 4, D], FP16, tag="xload", name=f"xs{c4}")
            # c4-1 splits into two 512KB DMAs so its first half can start
            # transposing ~5us earlier (c4-0 stays one DMA: nothing queues
            # ahead of it)
            nparts = 2 if c4 == 1 else 1
            for part in range(nparts):
                lo, hi = part * (4 // nparts), (part + 1) * (4 // nparts)
                nc.sync.dma_start(
                    out=xs[:, lo:hi, :],
                    in_=x[c4 * 512 + lo * 128:c4 * 512 + hi * 128,
                          :].rearrange("(r p) d -> p r d", p=128))
            for col in range(4):
                ti = c4 * 4 + col
                for k in range(N_KI):
                    pt = psum.tile([128, 128], FP16, tag="psB", bufs=3,
                                   name=f"psx{ti}_{k}")
                    nc.tensor.transpose(
                        out=pt,
                        in_=xs[:, col, k * 128:(k + 1) * 128],
                        identity=ident)
                    nc.vector.tensor_copy(
                        xT[k][c4][:, col * 128:(col + 1) * 128], pt)

        def emit_sides(hf):
            # transposes + A-side Strassen operands for half hf
            c4a, c4b = 2 * hf, 2 * hf + 1
            if hf == 0:
                emit_xpose(c4a)
            if hf == 0:
                # dependency-free PE filler while the second x chunk's DMA
                # lands (the c4b transposes would otherwise head-block the
                # PE queue)
                for i in range(48):
                    pt = psum.tile([128, 128], FP, tag="psB", bufs=3,
                                   name=f"wf{i}")
                    nc.tensor.matmul(pt, lhsT=wtile, rhs=wtile,
                                     start=True, stop=True)
            if hf == 0:
                emit_xpose(c4b)
            s = [[s_pool.tile([128, 512], FP16, tag=f"s{si}_{kj}",
                              name=f"s{si}_{hf}_{kj}") for kj in range(N_KJ)]
                 for si in (1, 2, 3, 4)]
            s1, s2, s3, s4 = s
            # s1 first: products needing it (M5) run before M6/M7/M3
            for kj in range(N_KJ):
                nc.vector.tensor_add(s1[kj], xT[kj][c4b], xT[4 + kj][c4b])
            for kj in range(N_KJ):
                nc.vector.tensor_sub(s2[kj], s1[kj], xT[kj][c4a])
            for kj in range(N_KJ):
                nc.vector.tensor_sub(s3[kj], xT[kj][c4a], xT[kj][c4b])
            for kj in range(N_KJ):
                nc.vector.tensor_sub(s4[kj], xT[4 + kj][c4a], s2[kj])
            return s1, s2, s3, s4

        wp_cache = {}

        def wpt(hf, ht):
            k = (hf, ht)
            if k not in wp_cache:
                wp = w1s_pool.tile([128, N_KJ, 7, 128], FP16, tag="w1s",
                                   name=f"wp{hf}_{ht}")
                nc.scalar.dma_start(out=wp, in_=w1sv[ht])
                wp_cache[k] = wp
            return wp_cache[k]

        wpt(0, 0)   # first weight chunk ahead of the x stream on scalar
        sides = emit_sides(0)
        emit_xpose(2)
        emit_xpose(3)
        for hf in range(2):
            c4a, c4b = 2 * hf, 2 * hf + 1
            s1, s2, s3, s4 = sides

            # rhs tiles per product (index 1..7), per kj
            rhs_of = {
                1: [xT[kj][c4a] for kj in range(N_KJ)],
                2: [xT[4 + kj][c4a] for kj in range(N_KJ)],
                3: s4,
                4: [xT[4 + kj][c4b] for kj in range(N_KJ)],
                5: s1,
                6: s2,
                7: s3,
            }

            yth = [yt_pool.tile([128, TH], FP16, tag=f"yth{ht}",
                                name=f"yth{hf}_{ht}") for ht in range(N_HK)]

            w2_cache = {}

            def w2t(dc, hg, key):
                k = (dc, hg, key)
                if k not in w2_cache:
                    wt = w2_pool.tile([128, 4, 512], FP16, tag="w2",
                                      name=f"w2t{hf}_{dc}_{hg}_{key}")
                    nc.scalar.dma_start(
                        out=wt,
                        in_=w2[hg * 512:(hg + 1) * 512,
                               dc * 512:(dc + 1) * 512].rearrange(
                                   "(r p) d -> p r d", p=128))
                    w2_cache[k] = wt
                return w2_cache[k]

            for ht in range(N_HT):
                if ht == 11:
                    w2t(0, 0, f"{hf}_0")   # prefetch first sweep chunk
                if ht == 14:
                    w2t(0, 1, f"{hf}_0")
                wp = wpt(hf, ht)

                def product(i_prod, nm):
                    mt = psum.tile([128, 512], FP, tag="psA",
                                   name=f"m{nm}_{hf}_{ht}")
                    for kj in range(N_KJ):
                        nc.tensor.matmul(
                            mt,
                            lhsT=wp[:, kj, i_prod - 1, :],
                            rhs=rhs_of[i_prod][kj],
                            start=(kj == 0), stop=(kj == N_KJ - 1))
                    return mt

                # product order chosen so the PSUM ring slots of the first
                # allocations free early (M1 via the copy, M6/M7 via u2/u3):
                # the next group's matmuls then never wait on this group's
                # DVE assembly tail
                m1 = product(1, "1")
                m6 = product(6, "6")
                m7 = product(7, "7")

                # M1 is read twice; copy to SBUF on ScalarE so every DVE
                # tensor_tensor touches at most one PSUM bank
                m1c = us_pool.tile([128, 512], FP, tag="us", name=f"m1c{hf}_{ht}")
                nc.scalar.activation(out=m1c, in_=m1,
                                     func=mybir.ActivationFunctionType.Copy,
                                     scale=1.0)
                u2 = us_pool.tile([128, 512], FP, tag="us", name=f"u2_{hf}_{ht}")
                u3 = us_pool.tile([128, 512], FP, tag="us", name=f"u3_{hf}_{ht}")
                nc.vector.tensor_add(u2, m1c, m6)
                nc.vector.tensor_add(u3, u2, m7)

                m2 = product(2, "2")
                c11 = cs_pool.tile([128, 512], FP16, tag="cs", name=f"c11_{hf}_{ht}")
                nc.vector.tensor_add(c11, m1c, m2)
                m4 = product(4, "4")
                c21 = cs_pool.tile([128, 512], FP16, tag="cs", name=f"c21_{hf}_{ht}")
                nc.vector.tensor_sub(c21, u3, m4)
                m5 = product(5, "5")
                u4 = us_pool.tile([128, 512], FP, tag="us", name=f"u4_{hf}_{ht}")
                nc.vector.tensor_add(u4, u2, m5)
                c22 = cs_pool.tile([128, 512], FP16, tag="cs", name=f"c22_{hf}_{ht}")
                nc.vector.tensor_add(c22, u3, m5)
                m3 = product(3, "3")
                c12 = cs_pool.tile([128, 512], FP16, tag="cs", name=f"c12_{hf}_{ht}")
                nc.vector.tensor_add(c12, u4, m3)

                # bias + relu -> yT half tiles
                nc.scalar.activation(out=yth[ht][:, 0:512], in_=c11,
                                     func=RELU, bias=b1t[:, ht:ht + 1], scale=1.0)
                nc.scalar.activation(out=yth[ht][:, 512:1024], in_=c21,
                                     func=RELU, bias=b1t[:, ht:ht + 1], scale=1.0)
                nc.scalar.activation(out=yth[16 + ht][:, 0:512], in_=c12,
                                     func=RELU,
                                     bias=b1t[:, 16 + ht:17 + ht], scale=1.0)
                nc.scalar.activation(out=yth[16 + ht][:, 512:1024], in_=c22,
                                     func=RELU,
                                     bias=b1t[:, 16 + ht:17 + ht], scale=1.0)

            if hf == 0:
                sides = emit_sides(1)

            # ---- FC2 for this half: full contraction in PSUM ----
            # one sweep per 512-wide output chunk; 8 concurrent PSUM banks.
            # The very last sweep is split into two 4-til half-sweeps so the
            # final stores overlap the remaining matmuls instead of draining
            # ~1MB after the PE goes idle.
            def sweep(dc, tils, key):
                pts = {}
                for j, til in enumerate(tils):
                    tag = "psA" if j < 5 else "psB"
                    kw = dict(bufs=3) if j >= 5 else {}
                    pts[til] = psum.tile([128, 512], FP, tag=tag,
                                         name=f"psfc2_{key}_{til}", **kw)
                for hg in range(N_HK // 4):
                    wt = w2t(dc, hg, key)
                    for r in range(4):
                        hk = hg * 4 + r
                        for til in tils:
                            nc.tensor.matmul(
                                pts[til],
                                lhsT=yth[hk][:, til * 128:(til + 1) * 128],
                                rhs=wt[:, r, :],
                                start=(hk == 0), stop=(hk == N_HK - 1))
                for til in tils:
                    os = os_pool.tile([128, 512], FP16, tag="os",
                                      name=f"os_{key}_{til}")
                    if B2_IS_ZERO and til % 2 == 0:
                        # bias is identically zero: plain psum->sbuf copy can
                        # run on ScalarE, halving the serial drain at the end
                        # of each sweep (DVE handles the other half)
                        nc.scalar.activation(
                            out=os, in_=pts[til],
                            func=mybir.ActivationFunctionType.Copy, scale=1.0)
                    else:
                        nc.vector.tensor_add(os, pts[til],
                                             b2b[:, dc * 512:(dc + 1) * 512])
                    ti_abs = hf * N_TIL + til
                    ring = nc.sync if til % 2 == 0 else nc.scalar
                    ring.dma_start(
                        out=out[ti_abs * 128:(ti_abs + 1) * 128,
                                dc * 512:(dc + 1) * 512],
                        in_=os)

            sweep(0, list(range(N_TIL)), f"{hf}_0")
            sweep(1, list(range(N_TIL)), f"{hf}_1")


def build_module(b2_zero):
    global B2_IS_ZERO
    B2_IS_ZERO = b2_zero
    nc = bacc.Bacc("TRN2", target_bir_lowering=False, debug=False)
    x = nc.dram_tensor("x", [T, D], FP16, kind="ExternalInput").ap()
    w1s = nc.dram_tensor("w1s", [16, 128 * 4 * 7 * 128], FP16,
                         kind="ExternalInput").ap()
    b1 = nc.dram_tensor("fc1_b", [1, H], FP, kind="ExternalInput").ap()
    w2 = nc.dram_tensor("fc2_w", [H, D], FP16, kind="ExternalInput").ap()
    b2 = nc.dram_tensor("fc2_b", [1, D], FP, kind="ExternalInput").ap()
    out = nc.dram_tensor("out", [T, D], FP16, kind="ExternalOutput").ap()
    with tile.TileContext(nc) as tc:
        _emit_kernel(tc, out, x, w1s, b1, w2, b2)
    nc.compile()
    return nc


_CACHED = None


def _host_w1s(w1_f32):
    """Host-side Strassen-Winograd B-operands: [B11,B21,B22,T4,T1,T2,T3]."""
    b11 = w1_f32[0:512, 0:2048]
    b12 = w1_f32[0:512, 2048:4096]
    b21 = w1_f32[512:1024, 0:2048]
    b22 = w1_f32[512:1024, 2048:4096]
    t1 = b12 - b11
    t2 = b22 - t1
    t3 = b22 - b12
    t4 = t2 - b21
    w = np.stack([b11, b21, b22, t4, t1, t2, t3], axis=1)  # [512, 7, 2048]
    # -> [ht, p, kj, i, h'] so each per-ht slice is one contiguous DMA
    w5 = w.reshape(4, 128, 7, 16, 128).transpose(3, 1, 0, 2, 4)
    return np.ascontiguousarray(
        w5.reshape(16, 128 * 4 * 7 * 128).astype(np.float16))


def kernel(x, fc1_w, fc1_b, fc2_w, fc2_b, _trace=False, _trace_cores=None):
    b2_zero = bool(np.all(np.asarray(fc2_b) == 0.0))
    global _CACHED
    if _CACHED is None or _CACHED[0] != b2_zero:
        _CACHED = (b2_zero, build_module(b2_zero))
    nc = _CACHED[1]

    x = np.ascontiguousarray(np.asarray(x, dtype=np.float32).astype(np.float16))
    fc1_w = np.asarray(fc1_w, dtype=np.float32)
    fc1_b = np.ascontiguousarray(np.asarray(fc1_b, dtype=np.float32))
    fc2_w = np.ascontiguousarray(
        np.asarray(fc2_w, dtype=np.float32).astype(np.float16))
    fc2_b = np.ascontiguousarray(np.asarray(fc2_b, dtype=np.float32))

    in_maps = [
        {
            "x": x[e],
            "w1s": _host_w1s(fc1_w[e]),
            "fc1_b": fc1_b[e],
            "fc2_w": fc2_w[e],
            "fc2_b": fc2_b[e],
        }
        for e in range(E)
    ]
    kw = {}
    if _trace:
        kw = dict(trace=True,
                  trace_cores=_trace_cores if _trace_cores is not None else [0])
    res = run_bass_kernel_spmd(nc, in_maps, core_ids=list(range(NCORES)), **kw)
    out = np.stack([res.results[e]["out"].astype(np.float32)
                    for e in range(E)], axis=0)
    if _trace:
        return out, res
    return out


# revision 19
# speedup vs baseline: 1.2158x; 1.0001x over previous
"""Expert-parallel batched-expert FFN kernel for Trainium2 — Strassen FC1.

Reference computation (per expert e):
    y = relu(x[e] @ fc1_w[e] + fc1_b[e]) @ fc2_w[e] + fc2_b[e]

Sharding: E=8 experts, one expert per core (expert parallel, no collectives).

Per-core algorithm (T=2048 tokens, D=1024, H=4096), fp16 operands:
  - Tokens are processed in two halves of 1024.  Within a half, FC1 is
    computed with one level of Strassen-Winograd: A = x-half [1024, 1024]
    split into [512, 512] blocks, B = w1 [1024, 4096] into [512, 2048]
    blocks.  The 7 B-side operands (B11, B21, B22, T4, T1, T2, T3) are
    precomputed on the host and streamed; the 4 A-side operands S1..S4 are
    built on the DVE from the transposed x tiles; the 7 products run on the
    PE (7/8 of the classic MAC count); the C-quadrant assembly (7 adds) runs
    on the DVE out of PSUM, then ScalarE applies bias+relu producing the
    fp16 yT half [4096, 1024] resident in SBUF.
  - FC2 for a half accumulates its full 4096-deep contraction in PSUM
    (no partial-sum traffic): two sweeps (one per 512-wide output column
    chunk) of 8 concurrent [128, 512] PSUM banks; w2 columns stream per
    sweep.  Output tiles get bias via one DVE add (or a ScalarE copy when
    the bias is identically zero) and store as fp16, split across the two
    HW DMA rings to shorten the final drain.
  - x transposes are hybrid: half 1 on the PE (fast, ramp-critical), half 2
    via the XBAR DMA-transpose issued at t~0 whose ~45GB/s latency hides
    fully under half 1's compute, costing zero PE/DVE work.
  - Warm-up uses real dependency-free matmuls (HAM ignores transposes);
    PSUM ring slots and Winograd product order are arranged so no group's
    matmuls ever wait on the previous group's DVE assembly tail.
"""

from contextlib import ExitStack

import numpy as np

import concourse.bass as bass
import concourse.bacc as bacc
import concourse.mybir as mybir
import concourse.tile as tile
from concourse.bass_utils import run_bass_kernel_spmd
from concourse.masks import make_identity

E, T, D, H = 8, 2048, 1024, 4096
NCORES = 8
TH = T // 2                    # tokens per half
FP = mybir.dt.float32
FP16 = mybir.dt.float16
RELU = mybir.ActivationFunctionType.Relu

N_KI = D // 128                # 8  k-tiles of x
N_KJ = 4                       # k-tiles per Strassen d-block (512)
N_HT = 16                      # h-tiles per Strassen h-block (2048)
N_C4 = T // 512                # 4  512-token chunks
N_HK = H // 128                # 32 h k-tiles for FC2
N_TIL = TH // 128              # 8  token tiles per half
N_DC = D // 512                # 2


B2_IS_ZERO = False


def _emit_kernel(tc, out, x, w1s, b1, w2, b2):
    nc = tc.nc
    with ExitStack() as ctx:
        singles = ctx.enter_context(tc.tile_pool(name="singles", bufs=1))
        xload = ctx.enter_context(tc.tile_pool(name="xload", bufs=2))
        xt_pool = ctx.enter_context(tc.tile_pool(name="xt", bufs=1))
        s_pool = ctx.enter_context(tc.tile_pool(name="spool", bufs=1))
        yt_pool = ctx.enter_context(tc.tile_pool(name="yt", bufs=1))
        w1s_pool = ctx.enter_context(tc.tile_pool(name="w1s", bufs=3))
        w2_pool = ctx.enter_context(tc.tile_pool(name="w2", bufs=3))
        us_pool = ctx.enter_context(tc.tile_pool(name="us", bufs=4))
        cs_pool = ctx.enter_context(tc.tile_pool(name="cs", bufs=5))
        os_pool = ctx.enter_context(tc.tile_pool(name="os", bufs=8))
        psum = ctx.enter_context(tc.tile_pool(name="psum", bufs=5, space="PSUM"))

        ident = singles.tile([128, 128], FP16)
        make_identity(nc, ident)

        # b1 [1, H] -> [128, H//128] with [p, hi] = b1[hi*128 + p]
        b1t = singles.tile([128, H // 128], FP)
        nc.scalar.dma_start(out=b1t, in_=b1.rearrange("o (h p) -> (o p) h", p=128))

        # b2 [1, D] broadcast across partitions -> [128, D]
        b2b = singles.tile([128, D], FP)
        b2_bcast = bass.AP(tensor=b2.tensor, offset=b2.offset,
                           ap=[[0, 128]] + [list(b2.ap[-1])])
        nc.scalar.dma_start(out=b2b, in_=b2_bcast)

        # w1s host layout [ht, p, kj, i, h'] -> per-ht loads are contiguous
        w1sv = w1s.rearrange("t (p r) -> t p r", p=128)

        # HAM warm-up with real matmuls
        wtile = singles.tile([128, 128], FP16)
        nc.vector.memset(wtile, 0.0)
        for i in range(56):
            pt = psum.tile([128, 128], FP, tag="psB", bufs=3, name=f"wu{i}")
            nc.tensor.matmul(pt, lhsT=wtile, rhs=wtile, start=True, stop=True)

        # x transposes, hybrid strategy:
        #  - half 1 (c4 0,1): PE transpose-mode (fast, needed immediately)
        #  - half 2 (c4 2,3): XBAR DMA-transpose (slow ~45GB/s, but issued at
        #    t~0 so its latency hides entirely under FC1 of half 1, costing
        #    zero PE/DVE work)
        xT = [[xt_pool.tile([128, 512], FP16, tag=f"xt{k}_{c4}",
                            name=f"xT{k}_{c4}")
               for c4 in range(2)] for k in range(N_KI)]
        xTc = [xt_pool.tile([128, N_KI, 512], FP16, tag=f"xtc{c4}",
                            name=f"xTc{c4}") for c4 in (2, 3)]
        for k in range(N_KI):
            xT[k].extend([xTc[0][:, k, :], xTc[1][:, k, :]])

        def emit_filler(n, nm):
            for i in range(n):
                pt = psum.tile([128, 128], FP, tag="psB", bufs=3,
                               name=f"wf{nm}_{i}")
                nc.tensor.matmul(pt, lhsT=wtile, rhs=wtile,
                                 start=True, stop=True)

        def emit_xpose(c4):
            if c4 >= 2:
                nc.sync.dma_start_transpose(
                    out=xTc[c4 - 2], in_=x[c4 * 512:(c4 + 1) * 512, :])
                return
            xs = xload.tile([128, 4, D], FP16, tag="xload", name=f"xs{c4}")
            # c4-1 splits into two 512KB DMAs so its first half can start
            # transposing ~5us earlier (c4-0 stays one DMA: nothing queues
            # ahead of it)
            nparts = 2 if c4 == 1 else 1
            for part in range(nparts):
                lo, hi = part * (4 // nparts), (part + 1) * (4 // nparts)
                nc.sync.dma_start(
                    out=xs[:, lo:hi, :],
                    in_=x[c4 * 512 + lo * 128:c4 * 512 + hi * 128,
                          :].rearrange("(r p) d -> p r d", p=128))
            for col in range(4):
                if c4 == 1 and col == 2:
                    # HAM keep-alive inside the part-2 DMA wait: the idle
                    # would otherwise trip the MID window and halve the PE
                    # clock for the next ~10us of real work
                    emit_filler(12, "mid")
                ti = c4 * 4 + col
                for k in range(N_KI):
                    pt = psum.tile([128, 128], FP16, tag="psB", bufs=3,
                                   name=f"psx{ti}_{k}")
                    nc.tensor.transpose(
                        out=pt,
                        in_=xs[:, col, k * 128:(k + 1) * 128],
                        identity=ident)
                    nc.vector.tensor_copy(
                        xT[k][c4][:, col * 128:(col + 1) * 128], pt)

        def emit_sides(hf):
            # transposes + A-side Strassen operands for half hf
            c4a, c4b = 2 * hf, 2 * hf + 1
            if hf == 0:
                emit_xpose(c4a)
            if hf == 0:
                # dependency-free PE filler while the second x chunk's DMA
                # lands (the c4b transposes would otherwise head-block the
                # PE queue)
                emit_filler(48, "pre")
            if hf == 0:
                emit_xpose(c4b)
            s = [[s_pool.tile([128, 512], FP16, tag=f"s{si}_{kj}",
                              name=f"s{si}_{hf}_{kj}") for kj in range(N_KJ)]
                 for si in (1, 2, 3, 4)]
            s1, s2, s3, s4 = s
            # s1 first: products needing it (M5) run before M6/M7/M3
            for kj in range(N_KJ):
                nc.vector.tensor_add(s1[kj], xT[kj][c4b], xT[4 + kj][c4b])
            for kj in range(N_KJ):
                nc.vector.tensor_sub(s2[kj], s1[kj], xT[kj][c4a])
            for kj in range(N_KJ):
                nc.vector.tensor_sub(s3[kj], xT[kj][c4a], xT[kj][c4b])
            for kj in range(N_KJ):
                nc.vector.tensor_sub(s4[kj], xT[4 + kj][c4a], s2[kj])
            return s1, s2, s3, s4

        wp_cache = {}

        def wpt(hf, ht):
            k = (hf, ht)
            if k not in wp_cache:
                wp = w1s_pool.tile([128, N_KJ, 7, 128], FP16, tag="w1s",
                                   name=f"wp{hf}_{ht}")
                nc.scalar.dma_start(out=wp, in_=w1sv[ht])
                wp_cache[k] = wp
            return wp_cache[k]

        wpt(0, 0)   # first weight chunk ahead of the x stream on scalar
        sides = emit_sides(0)
        emit_xpose(2)
        emit_xpose(3)
        for hf in range(2):
            c4a, c4b = 2 * hf, 2 * hf + 1
            s1, s2, s3, s4 = sides

            # rhs tiles per product (index 1..7), per kj
            rhs_of = {
                1: [xT[kj][c4a] for kj in range(N_KJ)],
                2: [xT[4 + kj][c4a] for kj in range(N_KJ)],
                3: s4,
                4: [xT[4 + kj][c4b] for kj in range(N_KJ)],
                5: s1,
                6: s2,
                7: s3,
            }

            yth = [yt_pool.tile([128, TH], FP16, tag=f"yth{ht}",
                                name=f"yth{hf}_{ht}") for ht in range(N_HK)]

            w2_cache = {}

            def w2t(dc, hg, key):
                k = (dc, hg, key)
                if k not in w2_cache:
                    wt = w2_pool.tile([128, 4, 512], FP16, tag="w2",
                                      name=f"w2t{hf}_{dc}_{hg}_{key}")
                    nc.scalar.dma_start(
                        out=wt,
                        in_=w2[hg * 512:(hg + 1) * 512,
                               dc * 512:(dc + 1) * 512].rearrange(
                                   "(r p) d -> p r d", p=128))
                    w2_cache[k] = wt
                return w2_cache[k]

            for ht in range(N_HT):
                if hf == 0 and ht in (1, 2, 3):
                    # HAM keep-alive through the early-group DMA waits
                    emit_filler(8, f"g{ht}")
                if ht == 11:
                    w2t(0, 0, f"{hf}_0")   # prefetch first sweep chunk
                if ht == 14:
                    w2t(0, 1, f"{hf}_0")
                wp = wpt(hf, ht)

                def product(i_prod, nm):
                    mt = psum.tile([128, 512], FP, tag="psA",
                                   name=f"m{nm}_{hf}_{ht}")
                    for kj in range(N_KJ):
                        nc.tensor.matmul(
                            mt,
                            lhsT=wp[:, kj, i_prod - 1, :],
                            rhs=rhs_of[i_prod][kj],
                            start=(kj == 0), stop=(kj == N_KJ - 1))
                    return mt

                # product order chosen so the PSUM ring slots of the first
                # allocations free early (M1 via the copy, M6/M7 via u2/u3):
                # the next group's matmuls then never wait on this group's
                # DVE assembly tail
                m1 = product(1, "1")
                m6 = product(6, "6")
                m7 = product(7, "7")

                # M1 is read twice; copy to SBUF on ScalarE so every DVE
                # tensor_tensor touches at most one PSUM bank
                m1c = us_pool.tile([128, 512], FP, tag="us", name=f"m1c{hf}_{ht}")
                nc.scalar.activation(out=m1c, in_=m1,
                                     func=mybir.ActivationFunctionType.Copy,
                                     scale=1.0)
                u2 = us_pool.tile([128, 512], FP, tag="us", name=f"u2_{hf}_{ht}")
                u3 = us_pool.tile([128, 512], FP, tag="us", name=f"u3_{hf}_{ht}")
                nc.vector.tensor_add(u2, m1c, m6)
                nc.vector.tensor_add(u3, u2, m7)

                m2 = product(2, "2")
                c11 = cs_pool.tile([128, 512], FP16, tag="cs", name=f"c11_{hf}_{ht}")
                nc.vector.tensor_add(c11, m1c, m2)
                m4 = product(4, "4")
                c21 = cs_pool.tile([128, 512], FP16, tag="cs", name=f"c21_{hf}_{ht}")
                nc.vector.tensor_sub(c21, u3, m4)
                m5 = product(5, "5")
                u4 = us_pool.tile([128, 512], FP, tag="us", name=f"u4_{hf}_{ht}")
                nc.vector.tensor_add(u4, u2, m5)
                c22 = cs_pool.tile([128, 512], FP16, tag="cs", name=f"c22_{hf}_{ht}")
                nc.vector.tensor_add(c22, u3, m5)
                m3 = product(3, "3")
                c12 = cs_pool.tile([128, 512], FP16, tag="cs", name=f"c12_{hf}_{ht}")
                nc.vector.tensor_add(c12, u4, m3)

                # bias + relu -> yT half tiles
                nc.scalar.activation(out=yth[ht][:, 0:512], in_=c11,
                                     func=RELU, bias=b1t[:, ht:ht + 1], scale=1.0)
                nc.scalar.activation(out=yth[ht][:, 512:1024], in_=c21,
                                     func=RELU, bias=b1t[:, ht:ht + 1], scale=1.0)
                nc.scalar.activation(out=yth[16 + ht][:, 0:512], in_=c12,
                                     func=RELU,
                                     bias=b1t[:, 16 + ht:17 + ht], scale=1.0)
                nc.scalar.activation(out=yth[16 + ht][:, 512:1024], in_=c22,
                                     func=RELU,
                                     bias=b1t[:, 16 + ht:17 + ht], scale=1.0)

            if hf == 0:
                sides = emit_sides(1)

            # ---- FC2 for this half: full contraction in PSUM ----
            # one sweep per 512-wide output chunk; 8 concurrent PSUM banks.
            # The very last sweep is split into two 4-til half-sweeps so the
            # final stores overlap the remaining matmuls instead of draining
            # ~1MB after the PE goes idle.
            def sweep(dc, tils, key):
                pts = {}
                for j, til in enumerate(tils):
                    tag = "psA" if j < 5 else "psB"
                    kw = dict(bufs=3) if j >= 5 else {}
                    pts[til] = psum.tile([128, 512], FP, tag=tag,
                                         name=f"psfc2_{key}_{til}", **kw)
                for hg in range(N_HK // 4):
                    wt = w2t(dc, hg, key)
                    for r in range(4):
                        hk = hg * 4 + r
                        for til in tils:
                            nc.tensor.matmul(
                                pts[til],
                                lhsT=yth[hk][:, til * 128:(til + 1) * 128],
                                rhs=wt[:, r, :],
                                start=(hk == 0), stop=(hk == N_HK - 1))
                for til in tils:
                    os = os_pool.tile([128, 512], FP16, tag="os",
                                      name=f"os_{key}_{til}")
                    if B2_IS_ZERO and til % 2 == 0:
                        # bias is identically zero: plain psum->sbuf copy can
                        # run on ScalarE, halving the serial drain at the end
                        # of each sweep (DVE handles the other half)
                        nc.scalar.activation(
                            out=os, in_=pts[til],
                            func=mybir.ActivationFunctionType.Copy, scale=1.0)
                    else:
                        nc.vector.tensor_add(os, pts[til],
                                             b2b[:, dc * 512:(dc + 1) * 512])
                    ti_abs = hf * N_TIL + til
                    ring = nc.sync if til % 2 == 0 else nc.scalar
                    ring.dma_start(
                        out=out[ti_abs * 128:(ti_abs + 1) * 128,
                                dc * 512:(dc + 1) * 512],
                        in_=os)

            sweep(0, list(range(N_TIL)), f"{hf}_0")
            sweep(1, list(range(N_TIL)), f"{hf}_1")


def build_module(b2_zero):
    global B2_IS_ZERO
    B2_IS_ZERO = b2_zero
    nc = bacc.Bacc("TRN2", target_bir_lowering=False, debug=False)
    x = nc.dram_tensor("x", [T, D], FP16, kind="ExternalInput").ap()
    w1s = nc.dram_tensor("w1s", [16, 128 * 4 * 7 * 128], FP16,
                         kind="ExternalInput").ap()
    b1 = nc.dram_tensor("fc1_b", [1, H], FP, kind="ExternalInput").ap()
    w2 = nc.dram_tensor("fc2_w", [H, D], FP16, kind="ExternalInput").ap()
    b2 = nc.dram_tensor("fc2_b", [1, D], FP, kind="ExternalInput").ap()
    out = nc.dram_tensor("out", [T, D], FP16, kind="ExternalOutput").ap()
    with tile.TileContext(nc) as tc:
        _emit_kernel(tc, out, x, w1s, b1, w2, b2)
    nc.compile()
    return nc


_CACHED = None


def _host_w1s(w1_f32):
    """Host-side Strassen-Winograd B-operands: [B11,B21,B22,T4,T1,T2,T3]."""
    b11 = w1_f32[0:512, 0:2048]
    b12 = w1_f32[0:512, 2048:4096]
    b21 = w1_f32[512:1024, 0:2048]
    b22 = w1_f32[512:1024, 2048:4096]
    t1 = b12 - b11
    t2 = b22 - t1
    t3 = b22 - b12
    t4 = t2 - b21
    w = np.stack([b11, b21, b22, t4, t1, t2, t3], axis=1)  # [512, 7, 2048]
    # -> [ht, p, kj, i, h'] so each per-ht slice is one contiguous DMA
    w5 = w.reshape(4, 128, 7, 16, 128).transpose(3, 1, 0, 2, 4)
    return np.ascontiguousarray(
        w5.reshape(16, 128 * 4 * 7 * 128).astype(np.float16))


def kernel(x, fc1_w, fc1_b, fc2_w, fc2_b, _trace=False, _trace_cores=None):
    b2_zero = bool(np.all(np.asarray(fc2_b) == 0.0))
    global _CACHED
    if _CACHED is None or _CACHED[0] != b2_zero:
        _CACHED = (b2_zero, build_module(b2_zero))
    nc = _CACHED[1]

    x = np.ascontiguousarray(np.asarray(x, dtype=np.float32).astype(np.float16))
    fc1_w = np.asarray(fc1_w, dtype=np.float32)
    fc1_b = np.ascontiguousarray(np.asarray(fc1_b, dtype=np.float32))
    fc2_w = np.ascontiguousarray(
        np.asarray(fc2_w, dtype=np.float32).astype(np.float16))
    fc2_b = np.ascontiguousarray(np.asarray(fc2_b, dtype=np.float32))

    in_maps = [
        {
            "x": x[e],
            "w1s": _host_w1s(fc1_w[e]),
            "fc1_b": fc1_b[e],
            "fc2_w": fc2_w[e],
            "fc2_b": fc2_b[e],
        }
        for e in range(E)
    ]
    kw = {}
    if _trace:
        kw = dict(trace=True,
                  trace_cores=_trace_cores if _trace_cores is not None else [0])
    res = run_bass_kernel_spmd(nc, in_maps, core_ids=list(range(NCORES)), **kw)
    out = np.stack([res.results[e]["out"].astype(np.float32)
                    for e in range(E)], axis=0)
    if _trace:
        return out, res
    return out
